# revision 14
# baseline (speedup 1.0000x reference)
"""Multi-head latent attention (MLA) on Trainium2 — 8-core SPMD Bass kernel.

Reference computation (fp32):
    Q  = X @ W_q.T           [B,S,1024] -> heads [B,H,S,256]
    Kc = X @ W_k.T           [B,S,256]  (shared across heads, MQA-style)
    Vc = X @ W_v.T           [B,S,256]
    P  = softmax(Q Kc^T / sqrt(256))
    Y  = concat_h(P Vc) @ W_o.T

Sharding: 8 cores = (batch b in {0,1}) x (query s-chunk in {0..3}).
Each core projects Q for its own 1024-token chunk, recomputes the (small,
shared) compressed Kc^T / Vc for the whole batch from X_b^T, runs attention
for all 4 heads over its queries, and writes its [1024, 1024] fp32 output
slice.  Host concatenates.  (An AllGather variant that shares Kc/Vc across
cores exists behind use_cc=True, but intra-chip collectives crash the
NRT runtime on this stack and the cost model predicts they lose anyway.)

All matmuls run in bf16 with fp32 PSUM accumulation; softmax runs in fp32 on
the scalar (ACT) engine.  Scores are computed transposed (keys on partitions)
so softmax-normalisation is deferred: the P^T @ Vc matmuls are unnormalised
and each head's output is scaled by 1/l (broadcast via a rank-1 matmul)
before the W_o projection.  Row sums l accumulate on the idle vector engine.

Measured numerics (CoreSim + HW): rel-fro err ~4.3e-3 vs fp32 reference.
"""

import numpy as np
import ml_dtypes
from contextlib import ExitStack

import concourse.bass as bass
import concourse.tile as tile
from concourse import bacc, bass_isa, mybir
from concourse.bass_utils import run_bass_kernel_spmd

# ---- problem constants (hardcoded; kernel.py must be self-contained) ----
B, S, DM = 2, 4096, 1024
H, DK, DKV = 4, 256, 256
NCORES = 8
CHUNKS = 4                # query chunks per batch
SQ = S // CHUNKS          # 1024 queries per core
SCALE = 1.0 / 16.0        # 1/sqrt(DK)

P = 128                   # partitions
NT = S // P               # 32 key tiles
NCT = DM // P             # 8 contraction tiles over the model dim
FD = 512                  # matmul moving free-dim chunk (one fp32 PSUM bank)
NSH = SQ // FD            # 2 query free-dim chunks

BF16 = mybir.dt.bfloat16
F32 = mybir.dt.float32
F8 = mybir.dt.float8e4
NPBF16 = ml_dtypes.bfloat16

# Scores in fp8(e4m3) DoubleRow: Q^T/K^T stored fp8 interleaved [P, 2, *], one
# matmul per (h, t, sh, chain) consuming both 128-deep dkv blocks at 0.5
# cycles/row.  Q additionally carries an fp8 residual chain (QR_CHAIN) --
# without it the fro error lands at 2.05e-2, just over the 2e-2 gate; with it
# 0.95e-2.  K uncompensated (its error contribution is only ~0.85%).
SCORES_FP8 = True
QR_CHAIN = True
# Projections (Q/K/V) as 3-term fp8 DoubleRow: X8 W8 + Xr W8 + X8 Wr, with the
# weights pre-scaled by SW=32 on the host so they clear e4m3's subnormal range
# (raw std 0.02 would flush the residual); the 1/(SW*SW) descale rides the
# softmax exp's scale and 1/SW rides the V copy.  12 DR matmuls replace 8 bf16
# matmuls per accumulation group: 0.75x PE time, +~0.05% error.
PROJ_FP8 = True
SW = 32.0
# lpart (row-sum partials) accumulated in bf16: all-2-byte operands get the
# DVE 2x path (150us -> ~75us DVE busy); costs ~0.05% on l via the 128-lane
# fp32 partition reduce that follows.
LPART_BF16 = True

# Use on-device AllGather to share Kc/Vc across the 4 cores of a batch
# (False recomputes them from the full X_b on every core).  AllGather is
# predicted slower by the cost model AND crashes NRT on this stack -> False.
USE_CC = False

# Tuned emission knobs (cost-model swept): PSUM banks 4+3+1 = 8.
ATTNV_INTERLEAVE = False
PS_SC_BUFS = 4     # scores/projection/W_o accumulators ([128,512] fp32 banks)
PS_OV_BUFS = 4     # attn@V accumulators (lrl bank freed by GPSIMD l-reduction)
PS_LRL_BUFS = 1    # l / 1-l broadcast pipeline (one bank, per-chunk)
LRL_POOL = "sc"    # l/RL PSUM pool unused when L_VIA_GPSIMD; don't allocate its bank
L_VIA_GPSIMD = True   # partition-sum+broadcast of l on the idle GPSIMD engine (-13.6us)
LRL_SPLIT = True
EXP_WIDE = False
PT_BUFS = 36       # 4 spare P^T slots beyond the 32 live -> smoother head overlap
LPOOL_BUFS = 2
YPOOL_BUFS = 2
RECIP_VIA_ACT = False
PROJ_COPY_DVE = False
EDGE_TRIM_HEAD = False   # finer first DMAs (all tiles): helps head, hurts middle -> off
EDGE_TRIM_HEAD1 = False  # split ONLY tile 0's xq/wqt DMAs (first matmul needs 160KB)
EDGE_TRIM_TAIL = True    # ship each 512-wide Y chunk as soon as copied
LDW_PAIR = False         # pairing same-lhsT matmuls: lowering emits LDW per matmul regardless -> no gain
PE_WARMUP_MMS = 0        # dummy warm-up matmuls: model shows +6.8us schedule cost > HAM benefit -> off


def _emit_full(tc: tile.TileContext, nc_io, use_cc, rep=0):
    """Emit the complete per-core program (projections + attention + W_o)."""
    nc = tc.nc
    AF = mybir.ActivationFunctionType
    y, xt, xq, wqt, wkt, wvt, wot = nc_io[:7]

    with ExitStack() as ctx:
        acts = ctx.enter_context(tc.tile_pool(name=f"acts{rep}", bufs=1))

        ps_sc = ctx.enter_context(tc.tile_pool(name=f"ps_sc{rep}", bufs=PS_SC_BUFS, space="PSUM"))
        ps_ov = ctx.enter_context(tc.tile_pool(name=f"ps_ov{rep}", bufs=PS_OV_BUFS, space="PSUM"))
        if LRL_POOL == "lrl":
            ps_lrl = ctx.enter_context(tc.tile_pool(name=f"ps_lrl{rep}", bufs=PS_LRL_BUFS, space="PSUM"))

        if SCORES_FP8:
            qt8_sb = [acts.tile([P, 2, SQ], F8, tag=f"qt8{h}", name=f"qt8_sb{h}")
                      for h in range(H)]
            kt8_sb = acts.tile([P, 2, S], F8, tag="kt8", name="kt8_sb")
            if QR_CHAIN:
                qtr8_sb = [acts.tile([P, 2, SQ], F8, tag=f"qtr8{h}",
                                     name=f"qtr8_sb{h}") for h in range(H)]
        else:
            qt_sb = [acts.tile([P, SQ], BF16, tag=f"qt{j}", name=f"qt_sb{j}") for j in range(NCT)]
            kt_sb = [acts.tile([P, S], BF16, tag=f"kt{j}", name=f"kt_sb{j}") for j in range(2)]
        vc_sb = [acts.tile([P, DKV], BF16, tag=f"vc{t}", name=f"vc_sb{t}") for t in range(NT)]
        ot_sb = [acts.tile([P, SQ], BF16, tag=f"ot{j}", name=f"ot_sb{j}") for j in range(NCT)]
        ones_col = acts.tile([P, 1], F32, tag="ones_col", name="ones_col")
        ones_row = acts.tile([1, P], F32, tag="ones_row", name="ones_row")
        nc.vector.memset(ones_col, 1.0)
        nc.vector.memset(ones_row, 1.0)

        if PE_WARMUP_MMS and rep == 0:
            # The PE is DMA-idle for the first ~4us; HW runs the first ~3.4us
            # of matmuls at half clock (HAM cold).  Burn that window on dummy
            # matmuls over memset data so the real projections start warm.
            warm_sb = acts.tile([P, FD], F32, tag="warm", name="warm_sb")
            nc.vector.memset(warm_sb, 0.0)
            warm_ps = ps_sc.tile([1, FD], F32, tag="sc", name="ps_warm")
            for w in range(PE_WARMUP_MMS):
                nc.tensor.matmul(warm_ps, ones_col, warm_sb, start=True, stop=True)

        # ---- phase P (projection inputs live only inside this block) ----
        with tc.tile_pool(name=f"loadin{rep}", bufs=1) as loadin:
          if PROJ_FP8:
            NPAIR = NCT // 2
            xq8_sb = [loadin.tile([P, 2, SQ], F8, tag=f"xq8{i}", name=f"xq8_sb{i}")
                      for i in range(NPAIR)]
            xqr_sb = [loadin.tile([P, 2, SQ], F8, tag=f"xqr{i}", name=f"xqr_sb{i}")
                      for i in range(NPAIR)]
            wq8_sb = [loadin.tile([P, 2, DM], F8, tag=f"wq8{i}", name=f"wq8_sb{i}")
                      for i in range(NPAIR)]
            wqr_sb = [loadin.tile([P, 2, DM], F8, tag=f"wqr{i}", name=f"wqr_sb{i}")
                      for i in range(NPAIR)]
            wk8_sb = [loadin.tile([P, 2, DKV], F8, tag=f"wk8{i}", name=f"wk8_sb{i}")
                      for i in range(NPAIR)]
            wkr_sb = [loadin.tile([P, 2, DKV], F8, tag=f"wkr{i}", name=f"wkr_sb{i}")
                      for i in range(NPAIR)]
            wv8_sb = [loadin.tile([P, 2, DKV], F8, tag=f"wv8{i}", name=f"wv8_sb{i}")
                      for i in range(NPAIR)]
            wvr_sb = [loadin.tile([P, 2, DKV], F8, tag=f"wvr{i}", name=f"wvr_sb{i}")
                      for i in range(NPAIR)]
            xt8_sb = [loadin.tile([P, 2, S], F8, tag=f"xt8{i}", name=f"xt8_sb{i}")
                      for i in range(NPAIR)]
            xtr_sb = [loadin.tile([P, 2, S], F8, tag=f"xtr{i}", name=f"xtr_sb{i}")
                      for i in range(NPAIR)]
            xq8, xqr, wq8, wqr, wk8, wkr, wv8, wvr, xt8, xtr = nc_io[7:]
            for ip in range(NPAIR):
                for u in range(2):
                    r = (2 * ip + u) * P
                    nc.sync.dma_start(out=xq8_sb[ip][:, u, :], in_=xq8[r:r + P, :])
                    nc.sync.dma_start(out=wq8_sb[ip][:, u, :], in_=wq8[r:r + P, :])
                    nc.sync.dma_start(out=xqr_sb[ip][:, u, :], in_=xqr[r:r + P, :])
                    nc.sync.dma_start(out=wqr_sb[ip][:, u, :], in_=wqr[r:r + P, :])
            for ip in range(NPAIR):
                for u in range(2):
                    r = (2 * ip + u) * P
                    nc.sync.dma_start(out=xt8_sb[ip][:, u, :], in_=xt8[r:r + P, :])
                    nc.sync.dma_start(out=wk8_sb[ip][:, u, :], in_=wk8[r:r + P, :])
                    nc.sync.dma_start(out=wv8_sb[ip][:, u, :], in_=wv8[r:r + P, :])
                    nc.sync.dma_start(out=xtr_sb[ip][:, u, :], in_=xtr[r:r + P, :])
                    nc.sync.dma_start(out=wkr_sb[ip][:, u, :], in_=wkr[r:r + P, :])
                    nc.sync.dma_start(out=wvr_sb[ip][:, u, :], in_=wvr[r:r + P, :])

            # Q^T for the local queries: 3-term fp8 chains.
            for j in range(NCT):
                for sh in range(NSH):
                    ps = ps_sc.tile([P, FD], F32, tag="sc", name="ps_qt")
                    for term, (wl, xl) in enumerate(
                            ((wq8_sb, xq8_sb), (wq8_sb, xqr_sb), (wqr_sb, xq8_sb))):
                        for ip in range(NPAIR):
                            nc.tensor.matmul(
                                ps, wl[ip][:, :, j * P:(j + 1) * P],
                                xl[ip][:, :, sh * FD:(sh + 1) * FD],
                                start=(term == 0 and ip == 0),
                                stop=(term == 2 and ip == NPAIR - 1),
                                perf_mode=mybir.MatmulPerfMode.DoubleRow)
                    qdst = qt8_sb[j // 2][:, j % 2, sh * FD:(sh + 1) * FD]
                    nc.scalar.activation(qdst, ps, AF.Copy)
                    if QR_CHAIN:
                        nc.vector.tensor_sub(
                            qtr8_sb[j // 2][:, j % 2, sh * FD:(sh + 1) * FD],
                            ps, qdst)

            # K^T full batch: 3-term fp8 chains.
            for j in range(2):
                for tch in range(S // FD):
                    ps = ps_sc.tile([P, FD], F32, tag="sc", name="ps_kt")
                    for term, (wl, xl) in enumerate(
                            ((wk8_sb, xt8_sb), (wk8_sb, xtr_sb), (wkr_sb, xt8_sb))):
                        for ip in range(NPAIR):
                            nc.tensor.matmul(
                                ps, wl[ip][:, :, j * P:(j + 1) * P],
                                xl[ip][:, :, tch * FD:(tch + 1) * FD],
                                start=(term == 0 and ip == 0),
                                stop=(term == 2 and ip == NPAIR - 1),
                                perf_mode=mybir.MatmulPerfMode.DoubleRow)
                    nc.scalar.activation(
                        kt8_sb[:, j, tch * FD:(tch + 1) * FD], ps, AF.Copy)

            # Vc full batch: 3-term fp8 chains; 1/SW descale on the copy.
            for t in range(NT):
                ps = ps_sc.tile([P, DKV], F32, tag="sc", name="ps_vc")
                for term, (xl, wl) in enumerate(
                        ((xt8_sb, wv8_sb), (xtr_sb, wv8_sb), (xt8_sb, wvr_sb))):
                    for ip in range(NPAIR):
                        nc.tensor.matmul(
                            ps, xl[ip][:, :, t * P:(t + 1) * P], wl[ip],
                            start=(term == 0 and ip == 0),
                            stop=(term == 2 and ip == NPAIR - 1),
                            perf_mode=mybir.MatmulPerfMode.DoubleRow)
                nc.scalar.activation(vc_sb[t], ps, AF.Copy, scale=1.0 / SW)
          else:
            xq_sb = [loadin.tile([P, SQ], BF16, tag=f"xq{i}", name=f"xq_sb{i}") for i in range(NCT)]
            wqt_sb = [loadin.tile([P, DM], BF16, tag=f"wq{i}", name=f"wqt_sb{i}") for i in range(NCT)]
            wkt_sb = [loadin.tile([P, DKV], BF16, tag=f"wk{i}", name=f"wkt_sb{i}") for i in range(NCT)]
            wvt_sb = [loadin.tile([P, DKV], BF16, tag=f"wv{i}", name=f"wvt_sb{i}") for i in range(NCT)]
            if EDGE_TRIM_HEAD:
                # First QT group (j=0, sh=0) only needs xq[:, :512] and
                # wqt[:, :128] of each c-tile — ship those first so the PE
                # starts ~2us sooner.
                for i in range(NCT):
                    nc.sync.dma_start(out=xq_sb[i][:, 0:FD], in_=xq[i * P:(i + 1) * P, 0:FD])
                    nc.sync.dma_start(out=wqt_sb[i][:, 0:P], in_=wqt[i * P:(i + 1) * P, 0:P])
                for i in range(NCT):
                    nc.sync.dma_start(out=xq_sb[i][:, FD:SQ], in_=xq[i * P:(i + 1) * P, FD:SQ])
                    nc.sync.dma_start(out=wqt_sb[i][:, P:DM], in_=wqt[i * P:(i + 1) * P, P:DM])
            elif EDGE_TRIM_HEAD1:
                nc.sync.dma_start(out=xq_sb[0][:, 0:FD], in_=xq[0:P, 0:FD])
                nc.sync.dma_start(out=wqt_sb[0][:, 0:P], in_=wqt[0:P, 0:P])
                nc.sync.dma_start(out=xq_sb[0][:, FD:SQ], in_=xq[0:P, FD:SQ])
                nc.sync.dma_start(out=wqt_sb[0][:, P:DM], in_=wqt[0:P, P:DM])
                for i in range(1, NCT):
                    nc.sync.dma_start(out=xq_sb[i], in_=xq[i * P:(i + 1) * P, :])
                    nc.sync.dma_start(out=wqt_sb[i], in_=wqt[i * P:(i + 1) * P, :])
            else:
                for i in range(NCT):
                    nc.sync.dma_start(out=xq_sb[i], in_=xq[i * P:(i + 1) * P, :])
                    nc.sync.dma_start(out=wqt_sb[i], in_=wqt[i * P:(i + 1) * P, :])
            if not use_cc:
                xt_sb = [loadin.tile([P, S], BF16, tag=f"xt{i}", name=f"xt_sb{i}")
                         for i in range(NCT)]
                for i in range(NCT):
                    nc.sync.dma_start(out=xt_sb[i], in_=xt[i * P:(i + 1) * P, :])
                    nc.sync.dma_start(out=wkt_sb[i], in_=wkt[i * P:(i + 1) * P, :])
                    nc.sync.dma_start(out=wvt_sb[i], in_=wvt[i * P:(i + 1) * P, :])
            else:
                for i in range(NCT):
                    nc.sync.dma_start(out=wkt_sb[i], in_=wkt[i * P:(i + 1) * P, :])
                    nc.sync.dma_start(out=wvt_sb[i], in_=wvt[i * P:(i + 1) * P, :])

            if use_cc:
                # -- K/V for the local chunk only, then AllGather over the batch --
                GROUPS = [[0, 1, 2, 3], [4, 5, 6, 7]]
                kc_slice = nc.dram_tensor(f"kc_slice{rep}", [DKV, SQ], BF16).ap()
                vc_slice = nc.dram_tensor(f"vc_slice{rep}", [SQ, DKV], BF16).ap()
                kc_ag = nc.dram_tensor(f"kc_ag{rep}", [CHUNKS, DKV, SQ], BF16).ap()
                vc_ag = nc.dram_tensor(f"vc_ag{rep}", [CHUNKS, SQ, DKV], BF16).ap()

                # Kc^T slice [DKV, SQ] from the local chunk columns (xq).
                for j in range(2):
                    ktloc = loadin.tile([P, SQ], BF16, tag=f"ktloc{j}", name=f"ktloc{j}")
                    for sh in range(NSH):
                        ps = ps_sc.tile([P, FD], F32, tag="sc", name="ps_kt")
                        for i in range(NCT):
                            nc.tensor.matmul(
                                ps, wkt_sb[i][:, j * P:(j + 1) * P],
                                xq_sb[i][:, sh * FD:(sh + 1) * FD],
                                start=(i == 0), stop=(i == NCT - 1))
                        nc.scalar.activation(ktloc[:, sh * FD:(sh + 1) * FD], ps, AF.Copy)
                    nc.sync.dma_start(out=kc_slice[j * P:(j + 1) * P, :], in_=ktloc)

                # Vc slice [SQ, DKV] from the local chunk.
                for tl in range(SQ // P):
                    vcloc = loadin.tile([P, DKV], BF16, tag="vcloc", name="vcloc", bufs=4)
                    ps = ps_sc.tile([P, DKV], F32, tag="sc", name="ps_vc")
                    for i in range(NCT):
                        nc.tensor.matmul(
                            ps, xq_sb[i][:, tl * P:(tl + 1) * P], wvt_sb[i],
                            start=(i == 0), stop=(i == NCT - 1))
                    nc.scalar.activation(vcloc, ps, AF.Copy)
                    nc.sync.dma_start(out=vc_slice[tl * P:(tl + 1) * P, :], in_=vcloc)

                nc.gpsimd.collective_compute(
                    "AllGather", mybir.AluOpType.bypass, replica_groups=GROUPS,
                    ins=[kc_slice], outs=[kc_ag])
                nc.gpsimd.collective_compute(
                    "AllGather", mybir.AluOpType.bypass, replica_groups=GROUPS,
                    ins=[vc_slice], outs=[vc_ag])

                # Load the gathered K/V back into SBUF.
                for j in range(2):
                    for r in range(CHUNKS):
                        nc.sync.dma_start(
                            out=kt_sb[j][:, r * SQ:(r + 1) * SQ],
                            in_=kc_ag[r, j * P:(j + 1) * P, :])
                for t in range(NT):
                    nc.sync.dma_start(
                        out=vc_sb[t], in_=vc_ag[t // 8, (t % 8) * P:(t % 8 + 1) * P, :])

            # Q^T for the local queries (overlaps the collective when use_cc).
            for j in range(NCT):
                for sh in range(NSH):
                    ps = ps_sc.tile([P, FD], F32, tag="sc", name="ps_qt")
                    for i in range(NCT):
                        nc.tensor.matmul(
                            ps, wqt_sb[i][:, j * P:(j + 1) * P],
                            xq_sb[i][:, sh * FD:(sh + 1) * FD],
                            start=(i == 0), stop=(i == NCT - 1))
                    qdst = (qt8_sb[j // 2][:, j % 2, sh * FD:(sh + 1) * FD]
                            if SCORES_FP8 else qt_sb[j][:, sh * FD:(sh + 1) * FD])
                    if PROJ_COPY_DVE:
                        nc.vector.tensor_copy(qdst, ps)
                    else:
                        nc.scalar.activation(qdst, ps, AF.Copy)

            if not use_cc:
                # -- recompute full-batch K/V on every core from xt --
                for j in range(2):
                    for tch in range(S // FD):
                        ps = ps_sc.tile([P, FD], F32, tag="sc", name="ps_kt")
                        for i in range(NCT):
                            nc.tensor.matmul(
                                ps, wkt_sb[i][:, j * P:(j + 1) * P],
                                xt_sb[i][:, tch * FD:(tch + 1) * FD],
                                start=(i == 0), stop=(i == NCT - 1))
                        kdst = (kt8_sb[:, j, tch * FD:(tch + 1) * FD]
                                if SCORES_FP8 else kt_sb[j][:, tch * FD:(tch + 1) * FD])
                        if PROJ_COPY_DVE:
                            nc.vector.tensor_copy(kdst, ps)
                        else:
                            nc.scalar.activation(kdst, ps, AF.Copy)
                for t in range(NT):
                    ps = ps_sc.tile([P, DKV], F32, tag="sc", name="ps_vc")
                    for i in range(NCT):
                        nc.tensor.matmul(
                            ps, xt_sb[i][:, t * P:(t + 1) * P], wvt_sb[i],
                            start=(i == 0), stop=(i == NCT - 1))
                    if PROJ_COPY_DVE:
                        nc.vector.tensor_copy(vc_sb[t], ps)
                    else:
                        nc.scalar.activation(vc_sb[t], ps, AF.Copy)

        # ---- attention phase (new pools reuse loadin's SBUF) ----
        attp = ctx.enter_context(tc.tile_pool(name=f"attp{rep}", bufs=1))
        pt_pool = ctx.enter_context(tc.tile_pool(name=f"pt{rep}", bufs=PT_BUFS))
        lpool = ctx.enter_context(tc.tile_pool(name=f"lpool{rep}", bufs=LPOOL_BUFS))
        ypool = ctx.enter_context(tc.tile_pool(name=f"ypool{rep}", bufs=YPOOL_BUFS))

        wot_sb = [attp.tile([P, DM], BF16, tag=f"wo{j}", name=f"wot_sb{j}") for j in range(NCT)]
        for j in range(NCT):
            nc.sync.dma_start(out=wot_sb[j], in_=wot[j * P:(j + 1) * P, :])

        for h in range(H):
            # scores^T + exp + row-sum partials
            lpart = lpool.tile([P, SQ], BF16 if LPART_BF16 else F32,
                               tag="lp", name="lpart")
            pt_tiles = []
            for t in range(NT):
                ptt = pt_pool.tile([P, SQ], BF16, tag="pt", name="pt_t")
                pt_tiles.append(ptt)
                if EXP_WIDE:
                    ps = ps_sc.tile([P, SQ], F32, tag="sc", name="ps_s")
                    for sh in range(NSH):
                        nc.tensor.matmul(
                            ps[:, sh * FD:(sh + 1) * FD],
                            kt_sb[0][:, t * P:(t + 1) * P],
                            qt_sb[2 * h][:, sh * FD:(sh + 1) * FD],
                            start=True, stop=False)
                        nc.tensor.matmul(
                            ps[:, sh * FD:(sh + 1) * FD],
                            kt_sb[1][:, t * P:(t + 1) * P],
                            qt_sb[2 * h + 1][:, sh * FD:(sh + 1) * FD],
                            start=False, stop=True)
                    nc.scalar.activation(ptt, ps, AF.Exp, scale=SCALE)
                elif LDW_PAIR:
                    pss = [ps_sc.tile([P, FD], F32, tag="sc", name="ps_s")
                           for _ in range(NSH)]
                    for kj in range(2):
                        for sh in range(NSH):
                            nc.tensor.matmul(
                                pss[sh], kt_sb[kj][:, t * P:(t + 1) * P],
                                qt_sb[2 * h + kj][:, sh * FD:(sh + 1) * FD],
                                start=(kj == 0), stop=(kj == 1),
                                skip_group_check=True)
                    for sh in range(NSH):
                        nc.scalar.activation(
                            ptt[:, sh * FD:(sh + 1) * FD], pss[sh], AF.Exp, scale=SCALE)
                elif SCORES_FP8:
                    escale = SCALE / (SW * SW) if PROJ_FP8 else SCALE
                    for sh in range(NSH):
                        ps = ps_sc.tile([P, FD], F32, tag="sc", name="ps_s")
                        nc.tensor.matmul(
                            ps, kt8_sb[:, :, t * P:(t + 1) * P],
                            qt8_sb[h][:, :, sh * FD:(sh + 1) * FD],
                            start=True, stop=not QR_CHAIN,
                            perf_mode=mybir.MatmulPerfMode.DoubleRow)
                        if QR_CHAIN:
                            nc.tensor.matmul(
                                ps, kt8_sb[:, :, t * P:(t + 1) * P],
                                qtr8_sb[h][:, :, sh * FD:(sh + 1) * FD],
                                start=False, stop=True,
                                perf_mode=mybir.MatmulPerfMode.DoubleRow)
                        nc.scalar.activation(
                            ptt[:, sh * FD:(sh + 1) * FD], ps, AF.Exp, scale=escale)
                else:
                    for sh in range(NSH):
                        ps = ps_sc.tile([P, FD], F32, tag="sc", name="ps_s")
                        nc.tensor.matmul(
                            ps, kt_sb[0][:, t * P:(t + 1) * P],
                            qt_sb[2 * h][:, sh * FD:(sh + 1) * FD],
                            start=True, stop=False)
                        nc.tensor.matmul(
                            ps, kt_sb[1][:, t * P:(t + 1) * P],
                            qt_sb[2 * h + 1][:, sh * FD:(sh + 1) * FD],
                            start=False, stop=True)
                        nc.scalar.activation(
                            ptt[:, sh * FD:(sh + 1) * FD], ps, AF.Exp, scale=SCALE)
                if t == 0:
                    nc.vector.tensor_copy(lpart, ptt)
                else:
                    nc.vector.tensor_add(lpart, lpart, ptt)

            # unnormalised attention output: O~^T[d, s] += Vc[t,d]^T P^T[t,s]
            if ATTNV_INTERLEAVE:
                # All four (sh, d-half) accumulators run in one t loop so
                # each PT tile is fully consumed at iteration t.
                ov_pairs = [
                    (ps_ov.tile([P, FD], F32, tag="ov", name="ps_ov0"),
                     ps_ov.tile([P, FD], F32, tag="ov", name="ps_ov1"))
                    for _ in range(NSH)
                ]
                for t in range(NT):
                    for dh in range(2):
                        for sh in range(NSH):
                            nc.tensor.matmul(
                                ov_pairs[sh][dh], vc_sb[t][:, dh * P:(dh + 1) * P],
                                pt_tiles[t][:, sh * FD:(sh + 1) * FD],
                                start=(t == 0), stop=(t == NT - 1))
            elif LDW_PAIR:
                ov_pairs = [
                    (ps_ov.tile([P, FD], F32, tag="ov", name="ps_ov0"),
                     ps_ov.tile([P, FD], F32, tag="ov", name="ps_ov1"))
                    for _ in range(NSH)
                ]
                for t in range(NT):
                    for dh in range(2):
                        for sh in range(NSH):
                            nc.tensor.matmul(
                                ov_pairs[sh][dh], vc_sb[t][:, dh * P:(dh + 1) * P],
                                pt_tiles[t][:, sh * FD:(sh + 1) * FD],
                                start=(t == 0), stop=(t == NT - 1),
                                skip_group_check=True)
            else:
                # One (sh) pair at a time: 2 live accumulators, 4 bufs ->
                # the pool double-buffers across s-chunks and heads.
                ov_pairs = []
                for sh in range(NSH):
                    ov0 = ps_ov.tile([P, FD], F32, tag="ov", name="ps_ov0")
                    ov1 = ps_ov.tile([P, FD], F32, tag="ov", name="ps_ov1")
                    ov_pairs.append((ov0, ov1))
                    for t in range(NT):
                        nc.tensor.matmul(
                            ov0, vc_sb[t][:, 0:P],
                            pt_tiles[t][:, sh * FD:(sh + 1) * FD],
                            start=(t == 0), stop=(t == NT - 1))
                        nc.tensor.matmul(
                            ov1, vc_sb[t][:, P:DKV],
                            pt_tiles[t][:, sh * FD:(sh + 1) * FD],
                            start=(t == 0), stop=(t == NT - 1))

            # l = sum_t P^T[t, s] (partition sum via ones matmul), rl = 1/l,
            # RL = broadcast of rl over 128 partitions (rank-1 matmul).
            rlb = lpool.tile([P, SQ], F32, tag="rlb", name="rlb")
            if L_VIA_GPSIMD:
                # GPSIMD does the partition sum AND the broadcast in one op,
                # freeing the PE matmuls and the l/RL PSUM bank.
                lbc = lpool.tile([P, SQ], F32, tag="lbc", name="lbc")
                nc.gpsimd.partition_all_reduce(
                    lbc, lpart, channels=P, reduce_op=bass_isa.ReduceOp.add)
                nc.vector.reciprocal(rlb, lbc)
            elif LRL_SPLIT:
                # one-bank l/RL pipeline, processed per s-chunk
                rl_row = lpool.tile([1, SQ], F32, tag="rl_row", name="rl_row")
                lrl_pool = {"lrl": ps_lrl if LRL_POOL == "lrl" else None,
                            "ov": ps_ov, "sc": ps_sc}[LRL_POOL]
                for sh in range(NSH):
                    l_ps = lrl_pool.tile([1, FD], F32, tag="sc" if LRL_POOL != "lrl" else "lrl", name="ps_l")
                    nc.tensor.matmul(
                        l_ps, ones_col, lpart[:, sh * FD:(sh + 1) * FD],
                        start=True, stop=True)
                    if RECIP_VIA_ACT:
                        # 1/l = exp(-ln l): both funcs live in the same ACT
                        # table set as the softmax exp -> no table swaps, and
                        # ~6x faster than the DVE iterative divide.
                        lnl = lpool.tile([1, FD], F32, tag="lnl", name="lnl")
                        nc.scalar.activation(lnl, l_ps, AF.Ln)
                        nc.scalar.activation(
                            rl_row[:, sh * FD:(sh + 1) * FD], lnl, AF.Exp,
                            scale=-1.0)
                    else:
                        nc.vector.reciprocal(rl_row[:, sh * FD:(sh + 1) * FD], l_ps)
                    rl_ps = lrl_pool.tile([P, FD], F32, tag="sc" if LRL_POOL != "lrl" else "lrl", name="ps_rl")
                    nc.tensor.matmul(
                        rl_ps, ones_row, rl_row[:, sh * FD:(sh + 1) * FD],
                        start=True, stop=True)
                    nc.scalar.activation(rlb[:, sh * FD:(sh + 1) * FD], rl_ps, AF.Copy)
            else:
                l_ps = ps_lrl.tile([1, SQ], F32, tag="lrl", name="ps_l")
                for sh in range(NSH):
                    nc.tensor.matmul(
                        l_ps[:, sh * FD:(sh + 1) * FD], ones_col,
                        lpart[:, sh * FD:(sh + 1) * FD], start=True, stop=True)
                rl_row = lpool.tile([1, SQ], F32, tag="rl_row", name="rl_row")
                nc.vector.reciprocal(rl_row, l_ps)
                rl_ps = ps_lrl.tile([P, SQ], F32, tag="lrl", name="ps_rl")
                for sh in range(NSH):
                    nc.tensor.matmul(
                        rl_ps[:, sh * FD:(sh + 1) * FD], ones_row,
                        rl_row[:, sh * FD:(sh + 1) * FD], start=True, stop=True)
                nc.scalar.activation(rlb, rl_ps, AF.Copy)

            # normalise while copying PSUM -> SBUF (bf16 for the W_o matmul)
            for sh in range(NSH):
                ov0, ov1 = ov_pairs[sh]
                nc.vector.tensor_mul(
                    ot_sb[2 * h][:, sh * FD:(sh + 1) * FD], ov0,
                    rlb[:, sh * FD:(sh + 1) * FD])
                nc.vector.tensor_mul(
                    ot_sb[2 * h + 1][:, sh * FD:(sh + 1) * FD], ov1,
                    rlb[:, sh * FD:(sh + 1) * FD])

        # ---- phase W: Y = O @ W_o^T ----
        for sb in range(SQ // P):
            ysb = ypool.tile([P, DM], F32, tag="y", name="ysb")
            for ec in range(DM // FD):
                ps = ps_sc.tile([P, FD], F32, tag="sc", name="ps_y")
                for j in range(NCT):
                    nc.tensor.matmul(
                        ps, ot_sb[j][:, sb * P:(sb + 1) * P],
                        wot_sb[j][:, ec * FD:(ec + 1) * FD],
                        start=(j == 0), stop=(j == NCT - 1))
                nc.scalar.activation(ysb[:, ec * FD:(ec + 1) * FD], ps, AF.Copy)
                if EDGE_TRIM_TAIL:
                    nc.sync.dma_start(
                        out=y[sb * P:(sb + 1) * P, ec * FD:(ec + 1) * FD],
                        in_=ysb[:, ec * FD:(ec + 1) * FD])
            if not EDGE_TRIM_TAIL:
                nc.sync.dma_start(out=y[sb * P:(sb + 1) * P, :], in_=ysb)


_BUILD_CACHE = {}


def build_program(use_cc=USE_CC, reps=1):
    """Build + compile the per-core Bass program (cached per process)."""
    key = ("nc", use_cc, reps)
    if key in _BUILD_CACHE:
        return _BUILD_CACHE[key]
    nc = bacc.Bacc("TRN2", target_bir_lowering=False, debug=False,
                   num_devices=NCORES)
    extra = ()
    if PROJ_FP8:
        assert SCORES_FP8 and not use_cc
        xt = xq = wqt = wkt = wvt = None
        extra = tuple(
            nc.dram_tensor(nm, shp, F8, kind="ExternalInput").ap()
            for nm, shp in (
                ("xq8", [DM, SQ]), ("xqr", [DM, SQ]),
                ("wq8", [DM, DM]), ("wqr", [DM, DM]),
                ("wk8", [DM, DKV]), ("wkr", [DM, DKV]),
                ("wv8", [DM, DKV]), ("wvr", [DM, DKV]),
                ("xt8", [DM, S]), ("xtr", [DM, S]),
            ))
    else:
        xt = (nc.dram_tensor("xt", [DM, S], BF16, kind="ExternalInput").ap()
              if not use_cc else None)
        xq = nc.dram_tensor("xq", [DM, SQ], BF16, kind="ExternalInput").ap()
        wqt = nc.dram_tensor("wqt", [DM, DM], BF16, kind="ExternalInput").ap()
        wkt = nc.dram_tensor("wkt", [DM, DKV], BF16, kind="ExternalInput").ap()
        wvt = nc.dram_tensor("wvt", [DM, DKV], BF16, kind="ExternalInput").ap()
    wot = nc.dram_tensor("wot", [DM, DM], BF16, kind="ExternalInput").ap()
    y = nc.dram_tensor("y", [SQ, DM], F32, kind="ExternalOutput").ap()
    with tile.TileContext(nc) as tc:
        for rep in range(reps):
            _emit_full(tc, (y, xt, xq, wqt, wkt, wvt, wot) + extra,
                       use_cc, rep=rep)
    nc.compile()
    _BUILD_CACHE[key] = nc
    return nc


def _split8(a):
    """fp32 array -> (fp8(a), fp8(a - fp8(a))) as float8_e4m3."""
    npf8 = mybir.dt.np(F8)
    hi = a.astype(npf8)
    lo = (a - hi.astype(np.float32)).astype(npf8)
    return hi, lo


def make_in_maps(X, W_q, W_k, W_v, W_o, use_cc=USE_CC):
    """Host-side shard prep: transpose + cast, one input dict per core."""
    wot = np.ascontiguousarray(W_o.T).astype(NPBF16)
    if PROJ_FP8:
        wq8, wqr = _split8(np.ascontiguousarray(W_q.T) * np.float32(SW))
        wk8, wkr = _split8(np.ascontiguousarray(W_k.T) * np.float32(SW))
        wv8, wvr = _split8(np.ascontiguousarray(W_v.T) * np.float32(SW))
        x8s = [_split8(np.ascontiguousarray(X[b].T)) for b in range(B)]
        in_maps = []
        for c in range(NCORES):
            b, chunk = divmod(c, CHUNKS)
            sl = slice(chunk * SQ, (chunk + 1) * SQ)
            in_maps.append({
                "xq8": np.ascontiguousarray(x8s[b][0][:, sl]),
                "xqr": np.ascontiguousarray(x8s[b][1][:, sl]),
                "wq8": wq8, "wqr": wqr, "wk8": wk8, "wkr": wkr,
                "wv8": wv8, "wvr": wvr,
                "xt8": x8s[b][0], "xtr": x8s[b][1], "wot": wot,
            })
        return in_maps
    wqt = np.ascontiguousarray(W_q.T).astype(NPBF16)
    wkt = np.ascontiguousarray(W_k.T).astype(NPBF16)
    wvt = np.ascontiguousarray(W_v.T).astype(NPBF16)
    xts = [np.ascontiguousarray(X[b].T).astype(NPBF16) for b in range(B)]
    in_maps = []
    for c in range(NCORES):
        b, chunk = divmod(c, CHUNKS)
        xq = np.ascontiguousarray(xts[b][:, chunk * SQ:(chunk + 1) * SQ])
        m = {"xq": xq, "wqt": wqt, "wkt": wkt, "wvt": wvt, "wot": wot}
        if not use_cc:
            m["xt"] = xts[b]
        in_maps.append(m)
    return in_maps


def run(X, W_q, W_k, W_v, W_o, trace=False, trace_cores=None, use_cc=USE_CC):
    """Run the 8-core kernel; returns (Y, BassKernelResults)."""
    nc = build_program(use_cc)
    in_maps = make_in_maps(X, W_q, W_k, W_v, W_o, use_cc)
    res = run_bass_kernel_spmd(
        nc, in_maps, list(range(NCORES)), trace=trace, trace_cores=trace_cores)
    Y = np.empty((B, S, DM), np.float32)
    for c in range(NCORES):
        b, chunk = divmod(c, CHUNKS)
        Y[b, chunk * SQ:(chunk + 1) * SQ, :] = res.results[c]["y"]
    return Y, res


def kernel(X, W_q, W_k, W_v, W_o):
    X = np.asarray(X)
    W_q = np.asarray(W_q)
    W_k = np.asarray(W_k)
    W_v = np.asarray(W_v)
    W_o = np.asarray(W_o)
    Y, _ = run(X, W_q, W_k, W_v, W_o)
    return Y



# revision 22
# speedup vs baseline: 1.3591x; 1.3591x over previous
"""Multi-head latent attention (MLA) on Trainium2 — 8-core SPMD Bass kernel.

Reference computation (fp32):
    Q  = X @ W_q.T           [B,S,1024] -> heads [B,H,S,256]
    Kc = X @ W_k.T           [B,S,256]  (shared across heads, MQA-style)
    Vc = X @ W_v.T           [B,S,256]
    P  = softmax(Q Kc^T / sqrt(256))
    Y  = concat_h(P Vc) @ W_o.T

Sharding: 8 cores = (batch b in {0,1}) x (query s-chunk in {0..3}).
Each core projects Q for its own 1024-token chunk, computes the compressed
Kc^T / Vc for that chunk only, AllGathers them across the 4 cores of its
batch (two ~0.5MB collectives, hidden behind the Q^T projection), runs
attention for all 4 heads over its queries, and writes its [1024, 1024]
fp32 output slice.  Host concatenates.  The collective path removes the
4x-redundant K/V recompute (-41us PE/core vs the use_cc=False variant,
which remains as an automatic fallback should the collective hit the rare
transient NRT worker crash observed once this session).

All matmuls run in bf16 with fp32 PSUM accumulation; softmax runs in fp32 on
the scalar (ACT) engine.  Scores are computed transposed (keys on partitions)
so softmax-normalisation is deferred: the P^T @ Vc matmuls are unnormalised
and each head's output is scaled by 1/l (broadcast via a rank-1 matmul)
before the W_o projection.  Row sums l accumulate on the idle vector engine.

Measured numerics (CoreSim + HW): rel-fro err ~4.3e-3 vs fp32 reference.
"""

import numpy as np
import ml_dtypes
from contextlib import ExitStack

import concourse.bass as bass
import concourse.tile as tile
from concourse import bacc, bass_isa, mybir
from concourse.bass_utils import run_bass_kernel_spmd

# ---- problem constants (hardcoded; kernel.py must be self-contained) ----
B, S, DM = 2, 4096, 1024
H, DK, DKV = 4, 256, 256
NCORES = 8
CHUNKS = 4                # query chunks per batch
SQ = S // CHUNKS          # 1024 queries per core
SCALE = 1.0 / 16.0        # 1/sqrt(DK)

P = 128                   # partitions
NT = S // P               # 32 key tiles
NCT = DM // P             # 8 contraction tiles over the model dim
FD = 512                  # matmul moving free-dim chunk (one fp32 PSUM bank)
NSH = SQ // FD            # 2 query free-dim chunks

BF16 = mybir.dt.bfloat16
F32 = mybir.dt.float32
F8 = mybir.dt.float8e4
NPBF16 = ml_dtypes.bfloat16

# Scores in fp8(e4m3) DoubleRow: Q^T/K^T stored fp8 interleaved [P, 2, *], one
# matmul per (h, t, sh, chain) consuming both 128-deep dkv blocks at 0.5
# cycles/row.  Q additionally carries an fp8 residual chain (QR_CHAIN) --
# without it the fro error lands at 2.05e-2, just over the 2e-2 gate; with it
# 0.95e-2.  K uncompensated (its error contribution is only ~0.85%).
# MEASURED (mm_bench, clean-cluster minima): a DoubleRow fp8 matmul costs the
# same per instruction as a bf16 matmul (~228ns at 512 free), i.e. 2x
# throughput per contraction pair -- NOT the cost model's 0.5 cyc/row (4x).
# Every numerically-viable fp8 config (Q-compensated scores = 2 chains,
# 3-term projections) therefore costs exactly what bf16 costs, and the
# 1-chain variants fail the 2e-2 gate (2.05e-2 scores; 2.00e-2 K-proj, where
# the shared weight-quantization error tilts scores coherently).  fp8 OFF.
SCORES_FP8 = False
QR_CHAIN = False
# Projections (Q/K/V) as 3-term fp8 DoubleRow: X8 W8 + Xr W8 + X8 Wr, with the
# weights pre-scaled by SW=32 on the host so they clear e4m3's subnormal range
# (raw std 0.02 would flush the residual); the 1/(SW*SW) descale rides the
# softmax exp's scale and 1/SW rides the V copy.  12 DR matmuls replace 8 bf16
# matmuls per accumulation group: 0.75x PE time, +~0.05% error.
PROJ_FP8 = False
SW = 32.0
# lpart (row-sum partials) accumulated in bf16: all-2-byte operands get the
# DVE 2x path (150us -> ~75us DVE busy); costs ~0.05% on l via the 128-lane
# fp32 partition reduce that follows.
LPART_BF16 = True

# Use on-device AllGather to share Kc/Vc across the 4 cores of a batch
# (False recomputes them from the full X_b on every core).  RE-TESTED this
# session (cc_probe.py / cc_probe2.py / abc_time.py): 5/6 fresh full-kernel
# runs + 2/2 probes + 1024 batched executions clean; one transient "worker
# hung up" on a first load, covered by the retry + no-CC fallback in run().
# Sharing removes the 4x-redundant K/V projections: measured -42us/dispatch
# vs the recompute variant (interleaved minima), matching the -41us
# cost-model prediction.
USE_CC = True

# Tuned emission knobs (cost-model swept): PSUM banks 4+3+1 = 8.
ATTNV_INTERLEAVE = False
PS_SC_BUFS = 4     # scores/projection/W_o accumulators ([128,512] fp32 banks)
PS_OV_BUFS = 4     # attn@V accumulators (lrl bank freed by GPSIMD l-reduction)
PS_LRL_BUFS = 1    # l / 1-l broadcast pipeline (one bank, per-chunk)
LRL_POOL = "sc"    # l/RL PSUM pool unused when L_VIA_GPSIMD; don't allocate its bank
L_VIA_GPSIMD = True   # partition-sum+broadcast of l on the idle GPSIMD engine (-13.6us)
LRL_SPLIT = True
EXP_WIDE = False
PT_BUFS = 36       # 4 spare P^T slots beyond the 32 live -> smoother head overlap
LPOOL_BUFS = 2
YPOOL_BUFS = 2
RECIP_VIA_ACT = False
PROJ_COPY_DVE = False
EDGE_TRIM_HEAD = False   # finer first DMAs (all tiles): helps head, hurts middle -> off
EDGE_TRIM_HEAD1 = False  # split ONLY tile 0's xq/wqt DMAs (first matmul needs 160KB)
EDGE_TRIM_TAIL = True    # ship each 512-wide Y chunk as soon as copied
LDW_PAIR = False         # pairing same-lhsT matmuls: lowering emits LDW per matmul regardless -> no gain
PE_WARMUP_MMS = 0        # dummy warm-up matmuls: model shows +6.8us schedule cost > HAM benefit -> off


def _emit_full(tc: tile.TileContext, nc_io, use_cc, rep=0):
    """Emit the complete per-core program (projections + attention + W_o)."""
    nc = tc.nc
    AF = mybir.ActivationFunctionType
    y, xt, xq, wqt, wkt, wvt, wot = nc_io[:7]

    with ExitStack() as ctx:
        acts = ctx.enter_context(tc.tile_pool(name=f"acts{rep}", bufs=1))

        ps_sc = ctx.enter_context(tc.tile_pool(name=f"ps_sc{rep}", bufs=PS_SC_BUFS, space="PSUM"))
        ps_ov = ctx.enter_context(tc.tile_pool(name=f"ps_ov{rep}", bufs=PS_OV_BUFS, space="PSUM"))
        if LRL_POOL == "lrl":
            ps_lrl = ctx.enter_context(tc.tile_pool(name=f"ps_lrl{rep}", bufs=PS_LRL_BUFS, space="PSUM"))

        if SCORES_FP8:
            qt8_sb = [acts.tile([P, 2, SQ], F8, tag=f"qt8{h}", name=f"qt8_sb{h}")
                      for h in range(H)]
            kt8_sb = acts.tile([P, 2, S], F8, tag="kt8", name="kt8_sb")
            if QR_CHAIN:
                qtr8_sb = [acts.tile([P, 2, SQ], F8, tag=f"qtr8{h}",
                                     name=f"qtr8_sb{h}") for h in range(H)]
        else:
            qt_sb = [acts.tile([P, SQ], BF16, tag=f"qt{j}", name=f"qt_sb{j}") for j in range(NCT)]
            kt_sb = [acts.tile([P, S], BF16, tag=f"kt{j}", name=f"kt_sb{j}") for j in range(2)]
        vc_sb = [acts.tile([P, DKV], BF16, tag=f"vc{t}", name=f"vc_sb{t}") for t in range(NT)]
        ot_sb = [acts.tile([P, SQ], BF16, tag=f"ot{j}", name=f"ot_sb{j}") for j in range(NCT)]
        ones_col = acts.tile([P, 1], F32, tag="ones_col", name="ones_col")
        ones_row = acts.tile([1, P], F32, tag="ones_row", name="ones_row")
        nc.vector.memset(ones_col, 1.0)
        nc.vector.memset(ones_row, 1.0)

        if PE_WARMUP_MMS and rep == 0:
            # The PE is DMA-idle for the first ~4us; HW runs the first ~3.4us
            # of matmuls at half clock (HAM cold).  Burn that window on dummy
            # matmuls over memset data so the real projections start warm.
            warm_sb = acts.tile([P, FD], F32, tag="warm", name="warm_sb")
            nc.vector.memset(warm_sb, 0.0)
            warm_ps = ps_sc.tile([1, FD], F32, tag="sc", name="ps_warm")
            for w in range(PE_WARMUP_MMS):
                nc.tensor.matmul(warm_ps, ones_col, warm_sb, start=True, stop=True)

        # ---- phase P (projection inputs live only inside this block) ----
        with tc.tile_pool(name=f"loadin{rep}", bufs=1) as loadin:
          if PROJ_FP8:
            NPAIR = NCT // 2
            xq8_sb = [loadin.tile([P, 2, SQ], F8, tag=f"xq8{i}", name=f"xq8_sb{i}")
                      for i in range(NPAIR)]
            xqr_sb = [loadin.tile([P, 2, SQ], F8, tag=f"xqr{i}", name=f"xqr_sb{i}")
                      for i in range(NPAIR)]
            wq8_sb = [loadin.tile([P, 2, DM], F8, tag=f"wq8{i}", name=f"wq8_sb{i}")
                      for i in range(NPAIR)]
            wqr_sb = [loadin.tile([P, 2, DM], F8, tag=f"wqr{i}", name=f"wqr_sb{i}")
                      for i in range(NPAIR)]
            wk8_sb = [loadin.tile([P, 2, DKV], F8, tag=f"wk8{i}", name=f"wk8_sb{i}")
                      for i in range(NPAIR)]
            wkr_sb = [loadin.tile([P, 2, DKV], F8, tag=f"wkr{i}", name=f"wkr_sb{i}")
                      for i in range(NPAIR)]
            wv8_sb = [loadin.tile([P, 2, DKV], F8, tag=f"wv8{i}", name=f"wv8_sb{i}")
                      for i in range(NPAIR)]
            wvr_sb = [loadin.tile([P, 2, DKV], F8, tag=f"wvr{i}", name=f"wvr_sb{i}")
                      for i in range(NPAIR)]
            xt8_sb = [loadin.tile([P, 2, S], F8, tag=f"xt8{i}", name=f"xt8_sb{i}")
                      for i in range(NPAIR)]
            xtr_sb = [loadin.tile([P, 2, S], F8, tag=f"xtr{i}", name=f"xtr_sb{i}")
                      for i in range(NPAIR)]
            xq8, xqr, wq8, wqr, wk8, wkr, wv8, wvr, xt8, xtr = nc_io[7:]
            for ip in range(NPAIR):
                for u in range(2):
                    r = (2 * ip + u) * P
                    nc.sync.dma_start(out=xq8_sb[ip][:, u, :], in_=xq8[r:r + P, :])
                    nc.sync.dma_start(out=wq8_sb[ip][:, u, :], in_=wq8[r:r + P, :])
                    nc.sync.dma_start(out=xqr_sb[ip][:, u, :], in_=xqr[r:r + P, :])
                    nc.sync.dma_start(out=wqr_sb[ip][:, u, :], in_=wqr[r:r + P, :])
            for ip in range(NPAIR):
                for u in range(2):
                    r = (2 * ip + u) * P
                    nc.sync.dma_start(out=xt8_sb[ip][:, u, :], in_=xt8[r:r + P, :])
                    nc.sync.dma_start(out=wk8_sb[ip][:, u, :], in_=wk8[r:r + P, :])
                    nc.sync.dma_start(out=wv8_sb[ip][:, u, :], in_=wv8[r:r + P, :])
                    nc.sync.dma_start(out=xtr_sb[ip][:, u, :], in_=xtr[r:r + P, :])
                    nc.sync.dma_start(out=wkr_sb[ip][:, u, :], in_=wkr[r:r + P, :])
                    nc.sync.dma_start(out=wvr_sb[ip][:, u, :], in_=wvr[r:r + P, :])

            # Q^T for the local queries: 3-term fp8 chains.
            for j in range(NCT):
                for sh in range(NSH):
                    ps = ps_sc.tile([P, FD], F32, tag="sc", name="ps_qt")
                    for term, (wl, xl) in enumerate(
                            ((wq8_sb, xq8_sb), (wq8_sb, xqr_sb), (wqr_sb, xq8_sb))):
                        for ip in range(NPAIR):
                            nc.tensor.matmul(
                                ps, wl[ip][:, :, j * P:(j + 1) * P],
                                xl[ip][:, :, sh * FD:(sh + 1) * FD],
                                start=(term == 0 and ip == 0),
                                stop=(term == 2 and ip == NPAIR - 1),
                                perf_mode=mybir.MatmulPerfMode.DoubleRow)
                    qdst = qt8_sb[j // 2][:, j % 2, sh * FD:(sh + 1) * FD]
                    nc.scalar.activation(qdst, ps, AF.Copy)
                    if QR_CHAIN:
                        nc.vector.tensor_sub(
                            qtr8_sb[j // 2][:, j % 2, sh * FD:(sh + 1) * FD],
                            ps, qdst)

            # K^T full batch: 3-term fp8 chains.
            for j in range(2):
                for tch in range(S // FD):
                    ps = ps_sc.tile([P, FD], F32, tag="sc", name="ps_kt")
                    for term, (wl, xl) in enumerate(
                            ((wk8_sb, xt8_sb), (wk8_sb, xtr_sb), (wkr_sb, xt8_sb))):
                        for ip in range(NPAIR):
                            nc.tensor.matmul(
                                ps, wl[ip][:, :, j * P:(j + 1) * P],
                                xl[ip][:, :, tch * FD:(tch + 1) * FD],
                                start=(term == 0 and ip == 0),
                                stop=(term == 2 and ip == NPAIR - 1),
                                perf_mode=mybir.MatmulPerfMode.DoubleRow)
                    nc.scalar.activation(
                        kt8_sb[:, j, tch * FD:(tch + 1) * FD], ps, AF.Copy)

            # Vc full batch: 3-term fp8 chains; 1/SW descale on the copy.
            for t in range(NT):
                ps = ps_sc.tile([P, DKV], F32, tag="sc", name="ps_vc")
                for term, (xl, wl) in enumerate(
                        ((xt8_sb, wv8_sb), (xtr_sb, wv8_sb), (xt8_sb, wvr_sb))):
                    for ip in range(NPAIR):
                        nc.tensor.matmul(
                            ps, xl[ip][:, :, t * P:(t + 1) * P], wl[ip],
                            start=(term == 0 and ip == 0),
                            stop=(term == 2 and ip == NPAIR - 1),
                            perf_mode=mybir.MatmulPerfMode.DoubleRow)
                nc.scalar.activation(vc_sb[t], ps, AF.Copy, scale=1.0 / SW)
          else:
            xq_sb = [loadin.tile([P, SQ], BF16, tag=f"xq{i}", name=f"xq_sb{i}") for i in range(NCT)]
            wqt_sb = [loadin.tile([P, DM], BF16, tag=f"wq{i}", name=f"wqt_sb{i}") for i in range(NCT)]
            wkt_sb = [loadin.tile([P, DKV], BF16, tag=f"wk{i}", name=f"wkt_sb{i}") for i in range(NCT)]
            wvt_sb = [loadin.tile([P, DKV], BF16, tag=f"wv{i}", name=f"wvt_sb{i}") for i in range(NCT)]
            if EDGE_TRIM_HEAD:
                # First QT group (j=0, sh=0) only needs xq[:, :512] and
                # wqt[:, :128] of each c-tile — ship those first so the PE
                # starts ~2us sooner.
                for i in range(NCT):
                    nc.sync.dma_start(out=xq_sb[i][:, 0:FD], in_=xq[i * P:(i + 1) * P, 0:FD])
                    nc.sync.dma_start(out=wqt_sb[i][:, 0:P], in_=wqt[i * P:(i + 1) * P, 0:P])
                for i in range(NCT):
                    nc.sync.dma_start(out=xq_sb[i][:, FD:SQ], in_=xq[i * P:(i + 1) * P, FD:SQ])
                    nc.sync.dma_start(out=wqt_sb[i][:, P:DM], in_=wqt[i * P:(i + 1) * P, P:DM])
            elif EDGE_TRIM_HEAD1:
                nc.sync.dma_start(out=xq_sb[0][:, 0:FD], in_=xq[0:P, 0:FD])
                nc.sync.dma_start(out=wqt_sb[0][:, 0:P], in_=wqt[0:P, 0:P])
                nc.sync.dma_start(out=xq_sb[0][:, FD:SQ], in_=xq[0:P, FD:SQ])
                nc.sync.dma_start(out=wqt_sb[0][:, P:DM], in_=wqt[0:P, P:DM])
                for i in range(1, NCT):
                    nc.sync.dma_start(out=xq_sb[i], in_=xq[i * P:(i + 1) * P, :])
                    nc.sync.dma_start(out=wqt_sb[i], in_=wqt[i * P:(i + 1) * P, :])
            else:
                for i in range(NCT):
                    nc.sync.dma_start(out=xq_sb[i], in_=xq[i * P:(i + 1) * P, :])
                    nc.sync.dma_start(out=wqt_sb[i], in_=wqt[i * P:(i + 1) * P, :])
            if not use_cc:
                xt_sb = [loadin.tile([P, S], BF16, tag=f"xt{i}", name=f"xt_sb{i}")
                         for i in range(NCT)]
                for i in range(NCT):
                    nc.sync.dma_start(out=xt_sb[i], in_=xt[i * P:(i + 1) * P, :])
                    nc.sync.dma_start(out=wkt_sb[i], in_=wkt[i * P:(i + 1) * P, :])
                    nc.sync.dma_start(out=wvt_sb[i], in_=wvt[i * P:(i + 1) * P, :])
            else:
                for i in range(NCT):
                    nc.sync.dma_start(out=wkt_sb[i], in_=wkt[i * P:(i + 1) * P, :])
                    nc.sync.dma_start(out=wvt_sb[i], in_=wvt[i * P:(i + 1) * P, :])

            if use_cc:
                # -- K/V for the local chunk only, then AllGather over the batch --
                GROUPS = [[0, 1, 2, 3], [4, 5, 6, 7]]
                kc_slice = nc.dram_tensor(f"kc_slice{rep}", [DKV, SQ], BF16).ap()
                vc_slice = nc.dram_tensor(f"vc_slice{rep}", [SQ, DKV], BF16).ap()
                kc_ag = nc.dram_tensor(f"kc_ag{rep}", [CHUNKS, DKV, SQ], BF16).ap()
                vc_ag = nc.dram_tensor(f"vc_ag{rep}", [CHUNKS, SQ, DKV], BF16).ap()

                # Kc^T slice [DKV, SQ] from the local chunk columns (xq).
                for j in range(2):
                    ktloc = loadin.tile([P, SQ], BF16, tag=f"ktloc{j}", name=f"ktloc{j}")
                    for sh in range(NSH):
                        ps = ps_sc.tile([P, FD], F32, tag="sc", name="ps_kt")
                        for i in range(NCT):
                            nc.tensor.matmul(
                                ps, wkt_sb[i][:, j * P:(j + 1) * P],
                                xq_sb[i][:, sh * FD:(sh + 1) * FD],
                                start=(i == 0), stop=(i == NCT - 1))
                        nc.scalar.activation(ktloc[:, sh * FD:(sh + 1) * FD], ps, AF.Copy)
                    nc.sync.dma_start(out=kc_slice[j * P:(j + 1) * P, :], in_=ktloc)

                # Vc slice [SQ, DKV] from the local chunk.
                for tl in range(SQ // P):
                    vcloc = loadin.tile([P, DKV], BF16, tag="vcloc", name="vcloc", bufs=4)
                    ps = ps_sc.tile([P, DKV], F32, tag="sc", name="ps_vc")
                    for i in range(NCT):
                        nc.tensor.matmul(
                            ps, xq_sb[i][:, tl * P:(tl + 1) * P], wvt_sb[i],
                            start=(i == 0), stop=(i == NCT - 1))
                    nc.scalar.activation(vcloc, ps, AF.Copy)
                    nc.sync.dma_start(out=vc_slice[tl * P:(tl + 1) * P, :], in_=vcloc)

                nc.gpsimd.collective_compute(
                    "AllGather", mybir.AluOpType.bypass, replica_groups=GROUPS,
                    ins=[kc_slice], outs=[kc_ag])
                nc.gpsimd.collective_compute(
                    "AllGather", mybir.AluOpType.bypass, replica_groups=GROUPS,
                    ins=[vc_slice], outs=[vc_ag])

                # Load the gathered K/V back into SBUF.
                for j in range(2):
                    for r in range(CHUNKS):
                        nc.sync.dma_start(
                            out=kt_sb[j][:, r * SQ:(r + 1) * SQ],
                            in_=kc_ag[r, j * P:(j + 1) * P, :])
                for t in range(NT):
                    nc.sync.dma_start(
                        out=vc_sb[t], in_=vc_ag[t // 8, (t % 8) * P:(t % 8 + 1) * P, :])

            # Q^T for the local queries (overlaps the collective when use_cc).
            for j in range(NCT):
                for sh in range(NSH):
                    ps = ps_sc.tile([P, FD], F32, tag="sc", name="ps_qt")
                    for i in range(NCT):
                        nc.tensor.matmul(
                            ps, wqt_sb[i][:, j * P:(j + 1) * P],
                            xq_sb[i][:, sh * FD:(sh + 1) * FD],
                            start=(i == 0), stop=(i == NCT - 1))
                    qdst = (qt8_sb[j // 2][:, j % 2, sh * FD:(sh + 1) * FD]
                            if SCORES_FP8 else qt_sb[j][:, sh * FD:(sh + 1) * FD])
                    if PROJ_COPY_DVE:
                        nc.vector.tensor_copy(qdst, ps)
                    else:
                        nc.scalar.activation(qdst, ps, AF.Copy)

            if not use_cc:
                # -- recompute full-batch K/V on every core from xt --
                for j in range(2):
                    for tch in range(S // FD):
                        ps = ps_sc.tile([P, FD], F32, tag="sc", name="ps_kt")
                        for i in range(NCT):
                            nc.tensor.matmul(
                                ps, wkt_sb[i][:, j * P:(j + 1) * P],
                                xt_sb[i][:, tch * FD:(tch + 1) * FD],
                                start=(i == 0), stop=(i == NCT - 1))
                        kdst = (kt8_sb[:, j, tch * FD:(tch + 1) * FD]
                                if SCORES_FP8 else kt_sb[j][:, tch * FD:(tch + 1) * FD])
                        if PROJ_COPY_DVE:
                            nc.vector.tensor_copy(kdst, ps)
                        else:
                            nc.scalar.activation(kdst, ps, AF.Copy)
                for t in range(NT):
                    ps = ps_sc.tile([P, DKV], F32, tag="sc", name="ps_vc")
                    for i in range(NCT):
                        nc.tensor.matmul(
                            ps, xt_sb[i][:, t * P:(t + 1) * P], wvt_sb[i],
                            start=(i == 0), stop=(i == NCT - 1))
                    if PROJ_COPY_DVE:
                        nc.vector.tensor_copy(vc_sb[t], ps)
                    else:
                        nc.scalar.activation(vc_sb[t], ps, AF.Copy)

        # ---- attention phase (new pools reuse loadin's SBUF) ----
        attp = ctx.enter_context(tc.tile_pool(name=f"attp{rep}", bufs=1))
        pt_pool = ctx.enter_context(tc.tile_pool(name=f"pt{rep}", bufs=PT_BUFS))
        lpool = ctx.enter_context(tc.tile_pool(name=f"lpool{rep}", bufs=LPOOL_BUFS))
        ypool = ctx.enter_context(tc.tile_pool(name=f"ypool{rep}", bufs=YPOOL_BUFS))

        wot_sb = [attp.tile([P, DM], BF16, tag=f"wo{j}", name=f"wot_sb{j}") for j in range(NCT)]
        for j in range(NCT):
            nc.sync.dma_start(out=wot_sb[j], in_=wot[j * P:(j + 1) * P, :])

        for h in range(H):
            # scores^T + exp + row-sum partials
            lpart = lpool.tile([P, SQ], BF16 if LPART_BF16 else F32,
                               tag="lp", name="lpart")
            pt_tiles = []
            for t in range(NT):
                ptt = pt_pool.tile([P, SQ], BF16, tag="pt", name="pt_t")
                pt_tiles.append(ptt)
                if EXP_WIDE:
                    ps = ps_sc.tile([P, SQ], F32, tag="sc", name="ps_s")
                    for sh in range(NSH):
                        nc.tensor.matmul(
                            ps[:, sh * FD:(sh + 1) * FD],
                            kt_sb[0][:, t * P:(t + 1) * P],
                            qt_sb[2 * h][:, sh * FD:(sh + 1) * FD],
                            start=True, stop=False)
                        nc.tensor.matmul(
                            ps[:, sh * FD:(sh + 1) * FD],
                            kt_sb[1][:, t * P:(t + 1) * P],
                            qt_sb[2 * h + 1][:, sh * FD:(sh + 1) * FD],
                            start=False, stop=True)
                    nc.scalar.activation(ptt, ps, AF.Exp, scale=SCALE)
                elif LDW_PAIR:
                    pss = [ps_sc.tile([P, FD], F32, tag="sc", name="ps_s")
                           for _ in range(NSH)]
                    for kj in range(2):
                        for sh in range(NSH):
                            nc.tensor.matmul(
                                pss[sh], kt_sb[kj][:, t * P:(t + 1) * P],
                                qt_sb[2 * h + kj][:, sh * FD:(sh + 1) * FD],
                                start=(kj == 0), stop=(kj == 1),
                                skip_group_check=True)
                    for sh in range(NSH):
                        nc.scalar.activation(
                            ptt[:, sh * FD:(sh + 1) * FD], pss[sh], AF.Exp, scale=SCALE)
                elif SCORES_FP8:
                    escale = SCALE / (SW * SW) if PROJ_FP8 else SCALE
                    for sh in range(NSH):
                        ps = ps_sc.tile([P, FD], F32, tag="sc", name="ps_s")
                        nc.tensor.matmul(
                            ps, kt8_sb[:, :, t * P:(t + 1) * P],
                            qt8_sb[h][:, :, sh * FD:(sh + 1) * FD],
                            start=True, stop=not QR_CHAIN,
                            perf_mode=mybir.MatmulPerfMode.DoubleRow)
                        if QR_CHAIN:
                            nc.tensor.matmul(
                                ps, kt8_sb[:, :, t * P:(t + 1) * P],
                                qtr8_sb[h][:, :, sh * FD:(sh + 1) * FD],
                                start=False, stop=True,
                                perf_mode=mybir.MatmulPerfMode.DoubleRow)
                        nc.scalar.activation(
                            ptt[:, sh * FD:(sh + 1) * FD], ps, AF.Exp, scale=escale)
                else:
                    for sh in range(NSH):
                        ps = ps_sc.tile([P, FD], F32, tag="sc", name="ps_s")
                        nc.tensor.matmul(
                            ps, kt_sb[0][:, t * P:(t + 1) * P],
                            qt_sb[2 * h][:, sh * FD:(sh + 1) * FD],
                            start=True, stop=False)
                        nc.tensor.matmul(
                            ps, kt_sb[1][:, t * P:(t + 1) * P],
                            qt_sb[2 * h + 1][:, sh * FD:(sh + 1) * FD],
                            start=False, stop=True)
                        nc.scalar.activation(
                            ptt[:, sh * FD:(sh + 1) * FD], ps, AF.Exp, scale=SCALE)
                if t == 0:
                    nc.vector.tensor_copy(lpart, ptt)
                else:
                    nc.vector.tensor_add(lpart, lpart, ptt)

            # unnormalised attention output: O~^T[d, s] += Vc[t,d]^T P^T[t,s]
            if ATTNV_INTERLEAVE:
                # All four (sh, d-half) accumulators run in one t loop so
                # each PT tile is fully consumed at iteration t.
                ov_pairs = [
                    (ps_ov.tile([P, FD], F32, tag="ov", name="ps_ov0"),
                     ps_ov.tile([P, FD], F32, tag="ov", name="ps_ov1"))
                    for _ in range(NSH)
                ]
                for t in range(NT):
                    for dh in range(2):
                        for sh in range(NSH):
                            nc.tensor.matmul(
                                ov_pairs[sh][dh], vc_sb[t][:, dh * P:(dh + 1) * P],
                                pt_tiles[t][:, sh * FD:(sh + 1) * FD],
                                start=(t == 0), stop=(t == NT - 1))
            elif LDW_PAIR:
                ov_pairs = [
                    (ps_ov.tile([P, FD], F32, tag="ov", name="ps_ov0"),
                     ps_ov.tile([P, FD], F32, tag="ov", name="ps_ov1"))
                    for _ in range(NSH)
                ]
                for t in range(NT):
                    for dh in range(2):
                        for sh in range(NSH):
                            nc.tensor.matmul(
                                ov_pairs[sh][dh], vc_sb[t][:, dh * P:(dh + 1) * P],
                                pt_tiles[t][:, sh * FD:(sh + 1) * FD],
                                start=(t == 0), stop=(t == NT - 1),
                                skip_group_check=True)
            else:
                # One (sh) pair at a time: 2 live accumulators, 4 bufs ->
                # the pool double-buffers across s-chunks and heads.
                ov_pairs = []
                for sh in range(NSH):
                    ov0 = ps_ov.tile([P, FD], F32, tag="ov", name="ps_ov0")
                    ov1 = ps_ov.tile([P, FD], F32, tag="ov", name="ps_ov1")
                    ov_pairs.append((ov0, ov1))
                    for t in range(NT):
                        nc.tensor.matmul(
                            ov0, vc_sb[t][:, 0:P],
                            pt_tiles[t][:, sh * FD:(sh + 1) * FD],
                            start=(t == 0), stop=(t == NT - 1))
                        nc.tensor.matmul(
                            ov1, vc_sb[t][:, P:DKV],
                            pt_tiles[t][:, sh * FD:(sh + 1) * FD],
                            start=(t == 0), stop=(t == NT - 1))

            # l = sum_t P^T[t, s] (partition sum via ones matmul), rl = 1/l,
            # RL = broadcast of rl over 128 partitions (rank-1 matmul).
            rlb = lpool.tile([P, SQ], F32, tag="rlb", name="rlb")
            if L_VIA_GPSIMD:
                # GPSIMD does the partition sum AND the broadcast in one op,
                # freeing the PE matmuls and the l/RL PSUM bank.
                lbc = lpool.tile([P, SQ], F32, tag="lbc", name="lbc")
                nc.gpsimd.partition_all_reduce(
                    lbc, lpart, channels=P, reduce_op=bass_isa.ReduceOp.add)
                nc.vector.reciprocal(rlb, lbc)
            elif LRL_SPLIT:
                # one-bank l/RL pipeline, processed per s-chunk
                rl_row = lpool.tile([1, SQ], F32, tag="rl_row", name="rl_row")
                lrl_pool = {"lrl": ps_lrl if LRL_POOL == "lrl" else None,
                            "ov": ps_ov, "sc": ps_sc}[LRL_POOL]
                for sh in range(NSH):
                    l_ps = lrl_pool.tile([1, FD], F32, tag="sc" if LRL_POOL != "lrl" else "lrl", name="ps_l")
                    nc.tensor.matmul(
                        l_ps, ones_col, lpart[:, sh * FD:(sh + 1) * FD],
                        start=True, stop=True)
                    if RECIP_VIA_ACT:
                        # 1/l = exp(-ln l): both funcs live in the same ACT
                        # table set as the softmax exp -> no table swaps, and
                        # ~6x faster than the DVE iterative divide.
                        lnl = lpool.tile([1, FD], F32, tag="lnl", name="lnl")
                        nc.scalar.activation(lnl, l_ps, AF.Ln)
                        nc.scalar.activation(
                            rl_row[:, sh * FD:(sh + 1) * FD], lnl, AF.Exp,
                            scale=-1.0)
                    else:
                        nc.vector.reciprocal(rl_row[:, sh * FD:(sh + 1) * FD], l_ps)
                    rl_ps = lrl_pool.tile([P, FD], F32, tag="sc" if LRL_POOL != "lrl" else "lrl", name="ps_rl")
                    nc.tensor.matmul(
                        rl_ps, ones_row, rl_row[:, sh * FD:(sh + 1) * FD],
                        start=True, stop=True)
                    nc.scalar.activation(rlb[:, sh * FD:(sh + 1) * FD], rl_ps, AF.Copy)
            else:
                l_ps = ps_lrl.tile([1, SQ], F32, tag="lrl", name="ps_l")
                for sh in range(NSH):
                    nc.tensor.matmul(
                        l_ps[:, sh * FD:(sh + 1) * FD], ones_col,
                        lpart[:, sh * FD:(sh + 1) * FD], start=True, stop=True)
                rl_row = lpool.tile([1, SQ], F32, tag="rl_row", name="rl_row")
                nc.vector.reciprocal(rl_row, l_ps)
                rl_ps = ps_lrl.tile([P, SQ], F32, tag="lrl", name="ps_rl")
                for sh in range(NSH):
                    nc.tensor.matmul(
                        rl_ps[:, sh * FD:(sh + 1) * FD], ones_row,
                        rl_row[:, sh * FD:(sh + 1) * FD], start=True, stop=True)
                nc.scalar.activation(rlb, rl_ps, AF.Copy)

            # normalise while copying PSUM -> SBUF (bf16 for the W_o matmul)
            for sh in range(NSH):
                ov0, ov1 = ov_pairs[sh]
                nc.vector.tensor_mul(
                    ot_sb[2 * h][:, sh * FD:(sh + 1) * FD], ov0,
                    rlb[:, sh * FD:(sh + 1) * FD])
                nc.vector.tensor_mul(
                    ot_sb[2 * h + 1][:, sh * FD:(sh + 1) * FD], ov1,
                    rlb[:, sh * FD:(sh + 1) * FD])

        # ---- phase W: Y = O @ W_o^T ----
        for sb in range(SQ // P):
            ysb = ypool.tile([P, DM], F32, tag="y", name="ysb")
            for ec in range(DM // FD):
                ps = ps_sc.tile([P, FD], F32, tag="sc", name="ps_y")
                for j in range(NCT):
                    nc.tensor.matmul(
                        ps, ot_sb[j][:, sb * P:(sb + 1) * P],
                        wot_sb[j][:, ec * FD:(ec + 1) * FD],
                        start=(j == 0), stop=(j == NCT - 1))
                nc.scalar.activation(ysb[:, ec * FD:(ec + 1) * FD], ps, AF.Copy)
                if EDGE_TRIM_TAIL:
                    nc.sync.dma_start(
                        out=y[sb * P:(sb + 1) * P, ec * FD:(ec + 1) * FD],
                        in_=ysb[:, ec * FD:(ec + 1) * FD])
            if not EDGE_TRIM_TAIL:
                nc.sync.dma_start(out=y[sb * P:(sb + 1) * P, :], in_=ysb)


_BUILD_CACHE = {}


def build_program(use_cc=USE_CC, reps=1):
    """Build + compile the per-core Bass program (cached per process)."""
    key = ("nc", use_cc, reps)
    if key in _BUILD_CACHE:
        return _BUILD_CACHE[key]
    nc = bacc.Bacc("TRN2", target_bir_lowering=False, debug=False,
                   num_devices=NCORES)
    extra = ()
    if PROJ_FP8:
        assert SCORES_FP8 and not use_cc
        xt = xq = wqt = wkt = wvt = None
        extra = tuple(
            nc.dram_tensor(nm, shp, F8, kind="ExternalInput").ap()
            for nm, shp in (
                ("xq8", [DM, SQ]), ("xqr", [DM, SQ]),
                ("wq8", [DM, DM]), ("wqr", [DM, DM]),
                ("wk8", [DM, DKV]), ("wkr", [DM, DKV]),
                ("wv8", [DM, DKV]), ("wvr", [DM, DKV]),
                ("xt8", [DM, S]), ("xtr", [DM, S]),
            ))
    else:
        xt = (nc.dram_tensor("xt", [DM, S], BF16, kind="ExternalInput").ap()
              if not use_cc else None)
        xq = nc.dram_tensor("xq", [DM, SQ], BF16, kind="ExternalInput").ap()
        wqt = nc.dram_tensor("wqt", [DM, DM], BF16, kind="ExternalInput").ap()
        wkt = nc.dram_tensor("wkt", [DM, DKV], BF16, kind="ExternalInput").ap()
        wvt = nc.dram_tensor("wvt", [DM, DKV], BF16, kind="ExternalInput").ap()
    wot = nc.dram_tensor("wot", [DM, DM], BF16, kind="ExternalInput").ap()
    y = nc.dram_tensor("y", [SQ, DM], F32, kind="ExternalOutput").ap()
    with tile.TileContext(nc) as tc:
        for rep in range(reps):
            _emit_full(tc, (y, xt, xq, wqt, wkt, wvt, wot) + extra,
                       use_cc, rep=rep)
    nc.compile()
    _BUILD_CACHE[key] = nc
    return nc


def _split8(a):
    """fp32 array -> (fp8(a), fp8(a - fp8(a))) as float8_e4m3."""
    npf8 = mybir.dt.np(F8)
    hi = a.astype(npf8)
    lo = (a - hi.astype(np.float32)).astype(npf8)
    return hi, lo


def make_in_maps(X, W_q, W_k, W_v, W_o, use_cc=USE_CC):
    """Host-side shard prep: transpose + cast, one input dict per core."""
    wot = np.ascontiguousarray(W_o.T).astype(NPBF16)
    if PROJ_FP8:
        wq8, wqr = _split8(np.ascontiguousarray(W_q.T) * np.float32(SW))
        wk8, wkr = _split8(np.ascontiguousarray(W_k.T) * np.float32(SW))
        wv8, wvr = _split8(np.ascontiguousarray(W_v.T) * np.float32(SW))
        x8s = [_split8(np.ascontiguousarray(X[b].T)) for b in range(B)]
        in_maps = []
        for c in range(NCORES):
            b, chunk = divmod(c, CHUNKS)
            sl = slice(chunk * SQ, (chunk + 1) * SQ)
            in_maps.append({
                "xq8": np.ascontiguousarray(x8s[b][0][:, sl]),
                "xqr": np.ascontiguousarray(x8s[b][1][:, sl]),
                "wq8": wq8, "wqr": wqr, "wk8": wk8, "wkr": wkr,
                "wv8": wv8, "wvr": wvr,
                "xt8": x8s[b][0], "xtr": x8s[b][1], "wot": wot,
            })
        return in_maps
    wqt = np.ascontiguousarray(W_q.T).astype(NPBF16)
    wkt = np.ascontiguousarray(W_k.T).astype(NPBF16)
    wvt = np.ascontiguousarray(W_v.T).astype(NPBF16)
    xts = [np.ascontiguousarray(X[b].T).astype(NPBF16) for b in range(B)]
    in_maps = []
    for c in range(NCORES):
        b, chunk = divmod(c, CHUNKS)
        xq = np.ascontiguousarray(xts[b][:, chunk * SQ:(chunk + 1) * SQ])
        m = {"xq": xq, "wqt": wqt, "wkt": wkt, "wvt": wvt, "wot": wot}
        if not use_cc:
            m["xt"] = xts[b]
        in_maps.append(m)
    return in_maps


def _run_once(X, W_q, W_k, W_v, W_o, trace, trace_cores, use_cc):
    nc = build_program(use_cc)
    in_maps = make_in_maps(X, W_q, W_k, W_v, W_o, use_cc)
    res = run_bass_kernel_spmd(
        nc, in_maps, list(range(NCORES)), trace=trace, trace_cores=trace_cores)
    Y = np.empty((B, S, DM), np.float32)
    for c in range(NCORES):
        b, chunk = divmod(c, CHUNKS)
        Y[b, chunk * SQ:(chunk + 1) * SQ, :] = res.results[c]["y"]
    return Y, res


def run(X, W_q, W_k, W_v, W_o, trace=False, trace_cores=None, use_cc=USE_CC):
    """Run the 8-core kernel; returns (Y, BassKernelResults).

    The AllGather K/V-sharing path (use_cc) very rarely hits a transient NRT
    worker crash on first load (observed once in ~6 fresh sessions); if that
    happens, retry once, then fall back to the collective-free variant that
    recomputes K/V per core.
    """
    if not use_cc:
        return _run_once(X, W_q, W_k, W_v, W_o, trace, trace_cores, False)
    for attempt in range(2):
        try:
            return _run_once(X, W_q, W_k, W_v, W_o, trace, trace_cores, True)
        except Exception as e:
            print(f"kernel: collective path failed ({type(e).__name__}: {e}); "
                  + ("retrying" if attempt == 0 else "falling back to no-CC"),
                  flush=True)
    return _run_once(X, W_q, W_k, W_v, W_o, trace, trace_cores, False)


def kernel(X, W_q, W_k, W_v, W_o):
    X = np.asarray(X)
    W_q = np.asarray(W_q)
    W_k = np.asarray(W_k)
    W_v = np.asarray(W_v)
    W_o = np.asarray(W_o)
    Y, _ = run(X, W_q, W_k, W_v, W_o)
    return Y



# revision 26
# speedup vs baseline: 1.5005x; 1.1041x over previous
"""Multi-head latent attention (MLA) on Trainium2 — 8-core SPMD Bass kernel.

Reference computation (fp32):
    Q  = X @ W_q.T           [B,S,1024] -> heads [B,H,S,256]
    Kc = X @ W_k.T           [B,S,256]  (shared across heads, MQA-style)
    Vc = X @ W_v.T           [B,S,256]
    P  = softmax(Q Kc^T / sqrt(256))
    Y  = concat_h(P Vc) @ W_o.T

Sharding: 8 cores = (batch b in {0,1}) x (query s-chunk in {0..3}).
Each core projects Q for its own 1024-token chunk, computes the compressed
Kc^T / Vc for that chunk only, AllGathers them across the 4 cores of its
batch (two ~0.5MB collectives, hidden behind the Q^T projection), runs
attention for all 4 heads over its queries, and writes its [1024, 1024]
fp32 output slice.  Host concatenates.  The collective path removes the
4x-redundant K/V recompute (-41us PE/core vs the use_cc=False variant,
which remains as an automatic fallback should the collective hit the rare
transient NRT worker crash observed once this session).

All matmuls run in bf16 with fp32 PSUM accumulation; softmax runs in fp32 on
the scalar (ACT) engine.  Scores are computed transposed (keys on partitions)
so softmax-normalisation is deferred: the P^T @ Vc matmuls are unnormalised
and each head's output is scaled by 1/l (broadcast via a rank-1 matmul)
before the W_o projection.  Row sums l accumulate on the idle vector engine.

Measured numerics (CoreSim + HW): rel-fro err ~4.3e-3 vs fp32 reference.
"""

import numpy as np
import ml_dtypes
from contextlib import ExitStack

import concourse.bass as bass
import concourse.tile as tile
from concourse import bacc, bass_isa, mybir
from concourse.bass_utils import run_bass_kernel_spmd

# ---- problem constants (hardcoded; kernel.py must be self-contained) ----
B, S, DM = 2, 4096, 1024
H, DK, DKV = 4, 256, 256
NCORES = 8
CHUNKS = 4                # query chunks per batch
SQ = S // CHUNKS          # 1024 queries per core
SCALE = 1.0 / 16.0        # 1/sqrt(DK)

P = 128                   # partitions
NT = S // P               # 32 key tiles
NCT = DM // P             # 8 contraction tiles over the model dim
FD = 512                  # matmul moving free-dim chunk (one fp32 PSUM bank)
NSH = SQ // FD            # 2 query free-dim chunks

BF16 = mybir.dt.bfloat16
F32 = mybir.dt.float32
F8 = mybir.dt.float8e4
NPBF16 = ml_dtypes.bfloat16

# Scores in fp8(e4m3) DoubleRow: Q^T/K^T stored fp8 interleaved [P, 2, *], one
# matmul per (h, t, sh, chain) consuming both 128-deep dkv blocks at 0.5
# cycles/row.  Q additionally carries an fp8 residual chain (QR_CHAIN) --
# without it the fro error lands at 2.05e-2, just over the 2e-2 gate; with it
# 0.95e-2.  K uncompensated (its error contribution is only ~0.85%).
# MEASURED (mm_bench, clean-cluster minima): a DoubleRow fp8 matmul costs the
# same per instruction as a bf16 matmul (~228ns at 512 free), i.e. 2x
# throughput per contraction pair -- NOT the cost model's 0.5 cyc/row (4x).
# Every numerically-viable fp8 config (Q-compensated scores = 2 chains,
# 3-term projections) therefore costs exactly what bf16 costs, and the
# 1-chain variants fail the 2e-2 gate (2.05e-2 scores; 2.00e-2 K-proj, where
# the shared weight-quantization error tilts scores coherently).  fp8 OFF.
SCORES_FP8 = False
QR_CHAIN = False
# Projections (Q/K/V) as 3-term fp8 DoubleRow: X8 W8 + Xr W8 + X8 Wr, with the
# weights pre-scaled by SW=32 on the host so they clear e4m3's subnormal range
# (raw std 0.02 would flush the residual); the 1/(SW*SW) descale rides the
# softmax exp's scale and 1/SW rides the V copy.  12 DR matmuls replace 8 bf16
# matmuls per accumulation group: 0.75x PE time, +~0.05% error.
PROJ_FP8 = False
SW = 32.0
# lpart (row-sum partials) accumulated in bf16: all-2-byte operands get the
# DVE 2x path (150us -> ~75us DVE busy); costs ~0.05% on l via the 128-lane
# fp32 partition reduce that follows.
LPART_BF16 = True

# Use on-device AllGather to share Kc/Vc across the 4 cores of a batch
# (False recomputes them from the full X_b on every core).  RE-TESTED this
# session (cc_probe.py / cc_probe2.py / abc_time.py): 5/6 fresh full-kernel
# runs + 2/2 probes + 1024 batched executions clean; one transient "worker
# hung up" on a first load, covered by the retry + no-CC fallback in run().
# Sharing removes the 4x-redundant K/V projections: measured -42us/dispatch
# vs the recompute variant (interleaved minima), matching the -41us
# cost-model prediction.
USE_CC = True
# Load-order for the CC path: wkt, xq halves, wvt, then wqt last, so the
# local-K projection (which gates the PE start AND the collective chain) is
# fed after ~1.5MB instead of ~5MB.
CC_DMA_ORDER = True

# Tuned emission knobs (cost-model swept): PSUM banks 4+3+1 = 8.
ATTNV_INTERLEAVE = False
PS_SC_BUFS = 4     # scores/projection/W_o accumulators ([128,512] fp32 banks)
PS_OV_BUFS = 4     # attn@V accumulators (lrl bank freed by GPSIMD l-reduction)
PS_LRL_BUFS = 1    # l / 1-l broadcast pipeline (one bank, per-chunk)
LRL_POOL = "sc"    # l/RL PSUM pool unused when L_VIA_GPSIMD; don't allocate its bank
L_VIA_GPSIMD = True   # partition-sum+broadcast of l on the idle GPSIMD engine (-13.6us)
LRL_SPLIT = True
EXP_WIDE = False
PT_BUFS = 36       # 4 spare P^T slots beyond the 32 live -> smoother head overlap
LPOOL_BUFS = 2
YPOOL_BUFS = 2
RECIP_VIA_ACT = False
PROJ_COPY_DVE = False
EDGE_TRIM_HEAD = False   # finer first DMAs (all tiles): helps head, hurts middle -> off
EDGE_TRIM_HEAD1 = False  # split ONLY tile 0's xq/wqt DMAs (first matmul needs 160KB)
EDGE_TRIM_TAIL = True    # ship each 512-wide Y chunk as soon as copied
LDW_PAIR = False         # pairing same-lhsT matmuls: lowering emits LDW per matmul regardless -> no gain
PE_WARMUP_MMS = 0        # dummy warm-up matmuls: model shows +6.8us schedule cost > HAM benefit -> off


def _emit_full(tc: tile.TileContext, nc_io, use_cc, rep=0):
    """Emit the complete per-core program (projections + attention + W_o)."""
    nc = tc.nc
    AF = mybir.ActivationFunctionType
    y, xt, xq, wqt, wkt, wvt, wot = nc_io[:7]

    with ExitStack() as ctx:
        acts = ctx.enter_context(tc.tile_pool(name=f"acts{rep}", bufs=1))

        ps_sc = ctx.enter_context(tc.tile_pool(name=f"ps_sc{rep}", bufs=PS_SC_BUFS, space="PSUM"))
        ps_ov = ctx.enter_context(tc.tile_pool(name=f"ps_ov{rep}", bufs=PS_OV_BUFS, space="PSUM"))
        if LRL_POOL == "lrl":
            ps_lrl = ctx.enter_context(tc.tile_pool(name=f"ps_lrl{rep}", bufs=PS_LRL_BUFS, space="PSUM"))

        if SCORES_FP8:
            qt8_sb = [acts.tile([P, 2, SQ], F8, tag=f"qt8{h}", name=f"qt8_sb{h}")
                      for h in range(H)]
            kt8_sb = acts.tile([P, 2, S], F8, tag="kt8", name="kt8_sb")
            if QR_CHAIN:
                qtr8_sb = [acts.tile([P, 2, SQ], F8, tag=f"qtr8{h}",
                                     name=f"qtr8_sb{h}") for h in range(H)]
        else:
            qt_sb = [acts.tile([P, SQ], BF16, tag=f"qt{j}", name=f"qt_sb{j}") for j in range(NCT)]
            kt_sb = [acts.tile([P, S], BF16, tag=f"kt{j}", name=f"kt_sb{j}") for j in range(2)]
        vc_sb = [acts.tile([P, DKV], BF16, tag=f"vc{t}", name=f"vc_sb{t}") for t in range(NT)]
        ot_sb = [acts.tile([P, SQ], BF16, tag=f"ot{j}", name=f"ot_sb{j}") for j in range(NCT)]
        ones_col = acts.tile([P, 1], F32, tag="ones_col", name="ones_col")
        ones_row = acts.tile([1, P], F32, tag="ones_row", name="ones_row")
        nc.vector.memset(ones_col, 1.0)
        nc.vector.memset(ones_row, 1.0)

        if PE_WARMUP_MMS and rep == 0:
            # The PE is DMA-idle for the first ~4us; HW runs the first ~3.4us
            # of matmuls at half clock (HAM cold).  Burn that window on dummy
            # matmuls over memset data so the real projections start warm.
            warm_sb = acts.tile([P, FD], F32, tag="warm", name="warm_sb")
            nc.vector.memset(warm_sb, 0.0)
            warm_ps = ps_sc.tile([1, FD], F32, tag="sc", name="ps_warm")
            for w in range(PE_WARMUP_MMS):
                nc.tensor.matmul(warm_ps, ones_col, warm_sb, start=True, stop=True)

        # ---- phase P (projection inputs live only inside this block) ----
        with tc.tile_pool(name=f"loadin{rep}", bufs=1) as loadin:
          if PROJ_FP8:
            NPAIR = NCT // 2
            xq8_sb = [loadin.tile([P, 2, SQ], F8, tag=f"xq8{i}", name=f"xq8_sb{i}")
                      for i in range(NPAIR)]
            xqr_sb = [loadin.tile([P, 2, SQ], F8, tag=f"xqr{i}", name=f"xqr_sb{i}")
                      for i in range(NPAIR)]
            wq8_sb = [loadin.tile([P, 2, DM], F8, tag=f"wq8{i}", name=f"wq8_sb{i}")
                      for i in range(NPAIR)]
            wqr_sb = [loadin.tile([P, 2, DM], F8, tag=f"wqr{i}", name=f"wqr_sb{i}")
                      for i in range(NPAIR)]
            wk8_sb = [loadin.tile([P, 2, DKV], F8, tag=f"wk8{i}", name=f"wk8_sb{i}")
                      for i in range(NPAIR)]
            wkr_sb = [loadin.tile([P, 2, DKV], F8, tag=f"wkr{i}", name=f"wkr_sb{i}")
                      for i in range(NPAIR)]
            wv8_sb = [loadin.tile([P, 2, DKV], F8, tag=f"wv8{i}", name=f"wv8_sb{i}")
                      for i in range(NPAIR)]
            wvr_sb = [loadin.tile([P, 2, DKV], F8, tag=f"wvr{i}", name=f"wvr_sb{i}")
                      for i in range(NPAIR)]
            xt8_sb = [loadin.tile([P, 2, S], F8, tag=f"xt8{i}", name=f"xt8_sb{i}")
                      for i in range(NPAIR)]
            xtr_sb = [loadin.tile([P, 2, S], F8, tag=f"xtr{i}", name=f"xtr_sb{i}")
                      for i in range(NPAIR)]
            xq8, xqr, wq8, wqr, wk8, wkr, wv8, wvr, xt8, xtr = nc_io[7:]
            for ip in range(NPAIR):
                for u in range(2):
                    r = (2 * ip + u) * P
                    nc.sync.dma_start(out=xq8_sb[ip][:, u, :], in_=xq8[r:r + P, :])
                    nc.sync.dma_start(out=wq8_sb[ip][:, u, :], in_=wq8[r:r + P, :])
                    nc.sync.dma_start(out=xqr_sb[ip][:, u, :], in_=xqr[r:r + P, :])
                    nc.sync.dma_start(out=wqr_sb[ip][:, u, :], in_=wqr[r:r + P, :])
            for ip in range(NPAIR):
                for u in range(2):
                    r = (2 * ip + u) * P
                    nc.sync.dma_start(out=xt8_sb[ip][:, u, :], in_=xt8[r:r + P, :])
                    nc.sync.dma_start(out=wk8_sb[ip][:, u, :], in_=wk8[r:r + P, :])
                    nc.sync.dma_start(out=wv8_sb[ip][:, u, :], in_=wv8[r:r + P, :])
                    nc.sync.dma_start(out=xtr_sb[ip][:, u, :], in_=xtr[r:r + P, :])
                    nc.sync.dma_start(out=wkr_sb[ip][:, u, :], in_=wkr[r:r + P, :])
                    nc.sync.dma_start(out=wvr_sb[ip][:, u, :], in_=wvr[r:r + P, :])

            # Q^T for the local queries: 3-term fp8 chains.
            for j in range(NCT):
                for sh in range(NSH):
                    ps = ps_sc.tile([P, FD], F32, tag="sc", name="ps_qt")
                    for term, (wl, xl) in enumerate(
                            ((wq8_sb, xq8_sb), (wq8_sb, xqr_sb), (wqr_sb, xq8_sb))):
                        for ip in range(NPAIR):
                            nc.tensor.matmul(
                                ps, wl[ip][:, :, j * P:(j + 1) * P],
                                xl[ip][:, :, sh * FD:(sh + 1) * FD],
                                start=(term == 0 and ip == 0),
                                stop=(term == 2 and ip == NPAIR - 1),
                                perf_mode=mybir.MatmulPerfMode.DoubleRow)
                    qdst = qt8_sb[j // 2][:, j % 2, sh * FD:(sh + 1) * FD]
                    nc.scalar.activation(qdst, ps, AF.Copy)
                    if QR_CHAIN:
                        nc.vector.tensor_sub(
                            qtr8_sb[j // 2][:, j % 2, sh * FD:(sh + 1) * FD],
                            ps, qdst)

            # K^T full batch: 3-term fp8 chains.
            for j in range(2):
                for tch in range(S // FD):
                    ps = ps_sc.tile([P, FD], F32, tag="sc", name="ps_kt")
                    for term, (wl, xl) in enumerate(
                            ((wk8_sb, xt8_sb), (wk8_sb, xtr_sb), (wkr_sb, xt8_sb))):
                        for ip in range(NPAIR):
                            nc.tensor.matmul(
                                ps, wl[ip][:, :, j * P:(j + 1) * P],
                                xl[ip][:, :, tch * FD:(tch + 1) * FD],
                                start=(term == 0 and ip == 0),
                                stop=(term == 2 and ip == NPAIR - 1),
                                perf_mode=mybir.MatmulPerfMode.DoubleRow)
                    nc.scalar.activation(
                        kt8_sb[:, j, tch * FD:(tch + 1) * FD], ps, AF.Copy)

            # Vc full batch: 3-term fp8 chains; 1/SW descale on the copy.
            for t in range(NT):
                ps = ps_sc.tile([P, DKV], F32, tag="sc", name="ps_vc")
                for term, (xl, wl) in enumerate(
                        ((xt8_sb, wv8_sb), (xtr_sb, wv8_sb), (xt8_sb, wvr_sb))):
                    for ip in range(NPAIR):
                        nc.tensor.matmul(
                            ps, xl[ip][:, :, t * P:(t + 1) * P], wl[ip],
                            start=(term == 0 and ip == 0),
                            stop=(term == 2 and ip == NPAIR - 1),
                            perf_mode=mybir.MatmulPerfMode.DoubleRow)
                nc.scalar.activation(vc_sb[t], ps, AF.Copy, scale=1.0 / SW)
          else:
            xq_sb = [loadin.tile([P, SQ], BF16, tag=f"xq{i}", name=f"xq_sb{i}") for i in range(NCT)]
            wqt_sb = [loadin.tile([P, DM], BF16, tag=f"wq{i}", name=f"wqt_sb{i}") for i in range(NCT)]
            wkt_sb = [loadin.tile([P, DKV], BF16, tag=f"wk{i}", name=f"wkt_sb{i}") for i in range(NCT)]
            wvt_sb = [loadin.tile([P, DKV], BF16, tag=f"wv{i}", name=f"wvt_sb{i}") for i in range(NCT)]
            if EDGE_TRIM_HEAD:
                # First QT group (j=0, sh=0) only needs xq[:, :512] and
                # wqt[:, :128] of each c-tile — ship those first so the PE
                # starts ~2us sooner.
                for i in range(NCT):
                    nc.sync.dma_start(out=xq_sb[i][:, 0:FD], in_=xq[i * P:(i + 1) * P, 0:FD])
                    nc.sync.dma_start(out=wqt_sb[i][:, 0:P], in_=wqt[i * P:(i + 1) * P, 0:P])
                for i in range(NCT):
                    nc.sync.dma_start(out=xq_sb[i][:, FD:SQ], in_=xq[i * P:(i + 1) * P, FD:SQ])
                    nc.sync.dma_start(out=wqt_sb[i][:, P:DM], in_=wqt[i * P:(i + 1) * P, P:DM])
            elif EDGE_TRIM_HEAD1:
                nc.sync.dma_start(out=xq_sb[0][:, 0:FD], in_=xq[0:P, 0:FD])
                nc.sync.dma_start(out=wqt_sb[0][:, 0:P], in_=wqt[0:P, 0:P])
                nc.sync.dma_start(out=xq_sb[0][:, FD:SQ], in_=xq[0:P, FD:SQ])
                nc.sync.dma_start(out=wqt_sb[0][:, P:DM], in_=wqt[0:P, P:DM])
                for i in range(1, NCT):
                    nc.sync.dma_start(out=xq_sb[i], in_=xq[i * P:(i + 1) * P, :])
                    nc.sync.dma_start(out=wqt_sb[i], in_=wqt[i * P:(i + 1) * P, :])
            elif use_cc and CC_DMA_ORDER:
                # kc local (the first PE work, and the gate for the whole
                # collective chain) needs wkt + the sh=0 halves of xq; QT's
                # wqt isn't touched until ~18us in.  Ship in need-order so
                # the PE starts at ~4us instead of ~14us.
                for i in range(NCT):
                    nc.sync.dma_start(out=wkt_sb[i], in_=wkt[i * P:(i + 1) * P, :])
                for i in range(NCT):
                    nc.sync.dma_start(out=xq_sb[i][:, 0:FD], in_=xq[i * P:(i + 1) * P, 0:FD])
                for i in range(NCT):
                    nc.sync.dma_start(out=wvt_sb[i], in_=wvt[i * P:(i + 1) * P, :])
                for i in range(NCT):
                    nc.sync.dma_start(out=xq_sb[i][:, FD:SQ], in_=xq[i * P:(i + 1) * P, FD:SQ])
                for i in range(NCT):
                    nc.sync.dma_start(out=wqt_sb[i], in_=wqt[i * P:(i + 1) * P, :])
            else:
                for i in range(NCT):
                    nc.sync.dma_start(out=xq_sb[i], in_=xq[i * P:(i + 1) * P, :])
                    nc.sync.dma_start(out=wqt_sb[i], in_=wqt[i * P:(i + 1) * P, :])
            if not use_cc:
                xt_sb = [loadin.tile([P, S], BF16, tag=f"xt{i}", name=f"xt_sb{i}")
                         for i in range(NCT)]
                for i in range(NCT):
                    nc.sync.dma_start(out=xt_sb[i], in_=xt[i * P:(i + 1) * P, :])
                    nc.sync.dma_start(out=wkt_sb[i], in_=wkt[i * P:(i + 1) * P, :])
                    nc.sync.dma_start(out=wvt_sb[i], in_=wvt[i * P:(i + 1) * P, :])
            elif not CC_DMA_ORDER:
                for i in range(NCT):
                    nc.sync.dma_start(out=wkt_sb[i], in_=wkt[i * P:(i + 1) * P, :])
                    nc.sync.dma_start(out=wvt_sb[i], in_=wvt[i * P:(i + 1) * P, :])

            if use_cc:
                # -- K/V for the local chunk only, then AllGather over the batch --
                GROUPS = [[0, 1, 2, 3], [4, 5, 6, 7]]
                kc_slice = nc.dram_tensor(f"kc_slice{rep}", [DKV, SQ], BF16).ap()
                vc_slice = nc.dram_tensor(f"vc_slice{rep}", [SQ, DKV], BF16).ap()
                kc_ag = nc.dram_tensor(f"kc_ag{rep}", [CHUNKS, DKV, SQ], BF16).ap()
                vc_ag = nc.dram_tensor(f"vc_ag{rep}", [CHUNKS, SQ, DKV], BF16).ap()

                # Kc^T slice [DKV, SQ] from the local chunk columns (xq).
                for j in range(2):
                    ktloc = loadin.tile([P, SQ], BF16, tag=f"ktloc{j}", name=f"ktloc{j}")
                    for sh in range(NSH):
                        ps = ps_sc.tile([P, FD], F32, tag="sc", name="ps_kt")
                        for i in range(NCT):
                            nc.tensor.matmul(
                                ps, wkt_sb[i][:, j * P:(j + 1) * P],
                                xq_sb[i][:, sh * FD:(sh + 1) * FD],
                                start=(i == 0), stop=(i == NCT - 1))
                        nc.scalar.activation(ktloc[:, sh * FD:(sh + 1) * FD], ps, AF.Copy)
                    nc.sync.dma_start(out=kc_slice[j * P:(j + 1) * P, :], in_=ktloc)

                # Vc slice [SQ, DKV] from the local chunk.
                for tl in range(SQ // P):
                    vcloc = loadin.tile([P, DKV], BF16, tag="vcloc", name="vcloc", bufs=4)
                    ps = ps_sc.tile([P, DKV], F32, tag="sc", name="ps_vc")
                    for i in range(NCT):
                        nc.tensor.matmul(
                            ps, xq_sb[i][:, tl * P:(tl + 1) * P], wvt_sb[i],
                            start=(i == 0), stop=(i == NCT - 1))
                    nc.scalar.activation(vcloc, ps, AF.Copy)
                    nc.sync.dma_start(out=vc_slice[tl * P:(tl + 1) * P, :], in_=vcloc)

                nc.gpsimd.collective_compute(
                    "AllGather", mybir.AluOpType.bypass, replica_groups=GROUPS,
                    ins=[kc_slice], outs=[kc_ag])
                nc.gpsimd.collective_compute(
                    "AllGather", mybir.AluOpType.bypass, replica_groups=GROUPS,
                    ins=[vc_slice], outs=[vc_ag])

                # Load the gathered K/V back into SBUF.
                for j in range(2):
                    for r in range(CHUNKS):
                        nc.sync.dma_start(
                            out=kt_sb[j][:, r * SQ:(r + 1) * SQ],
                            in_=kc_ag[r, j * P:(j + 1) * P, :])
                for t in range(NT):
                    nc.sync.dma_start(
                        out=vc_sb[t], in_=vc_ag[t // 8, (t % 8) * P:(t % 8 + 1) * P, :])

            # Q^T for the local queries (overlaps the collective when use_cc).
            for j in range(NCT):
                for sh in range(NSH):
                    ps = ps_sc.tile([P, FD], F32, tag="sc", name="ps_qt")
                    for i in range(NCT):
                        nc.tensor.matmul(
                            ps, wqt_sb[i][:, j * P:(j + 1) * P],
                            xq_sb[i][:, sh * FD:(sh + 1) * FD],
                            start=(i == 0), stop=(i == NCT - 1))
                    qdst = (qt8_sb[j // 2][:, j % 2, sh * FD:(sh + 1) * FD]
                            if SCORES_FP8 else qt_sb[j][:, sh * FD:(sh + 1) * FD])
                    if PROJ_COPY_DVE:
                        nc.vector.tensor_copy(qdst, ps)
                    else:
                        nc.scalar.activation(qdst, ps, AF.Copy)

            if not use_cc:
                # -- recompute full-batch K/V on every core from xt --
                for j in range(2):
                    for tch in range(S // FD):
                        ps = ps_sc.tile([P, FD], F32, tag="sc", name="ps_kt")
                        for i in range(NCT):
                            nc.tensor.matmul(
                                ps, wkt_sb[i][:, j * P:(j + 1) * P],
                                xt_sb[i][:, tch * FD:(tch + 1) * FD],
                                start=(i == 0), stop=(i == NCT - 1))
                        kdst = (kt8_sb[:, j, tch * FD:(tch + 1) * FD]
                                if SCORES_FP8 else kt_sb[j][:, tch * FD:(tch + 1) * FD])
                        if PROJ_COPY_DVE:
                            nc.vector.tensor_copy(kdst, ps)
                        else:
                            nc.scalar.activation(kdst, ps, AF.Copy)
                for t in range(NT):
                    ps = ps_sc.tile([P, DKV], F32, tag="sc", name="ps_vc")
                    for i in range(NCT):
                        nc.tensor.matmul(
                            ps, xt_sb[i][:, t * P:(t + 1) * P], wvt_sb[i],
                            start=(i == 0), stop=(i == NCT - 1))
                    if PROJ_COPY_DVE:
                        nc.vector.tensor_copy(vc_sb[t], ps)
                    else:
                        nc.scalar.activation(vc_sb[t], ps, AF.Copy)

        # ---- attention phase (new pools reuse loadin's SBUF) ----
        attp = ctx.enter_context(tc.tile_pool(name=f"attp{rep}", bufs=1))
        pt_pool = ctx.enter_context(tc.tile_pool(name=f"pt{rep}", bufs=PT_BUFS))
        lpool = ctx.enter_context(tc.tile_pool(name=f"lpool{rep}", bufs=LPOOL_BUFS))
        ypool = ctx.enter_context(tc.tile_pool(name=f"ypool{rep}", bufs=YPOOL_BUFS))

        wot_sb = [attp.tile([P, DM], BF16, tag=f"wo{j}", name=f"wot_sb{j}") for j in range(NCT)]
        for j in range(NCT):
            nc.sync.dma_start(out=wot_sb[j], in_=wot[j * P:(j + 1) * P, :])

        for h in range(H):
            # scores^T + exp + row-sum partials
            lpart = lpool.tile([P, SQ], BF16 if LPART_BF16 else F32,
                               tag="lp", name="lpart")
            pt_tiles = []
            for t in range(NT):
                ptt = pt_pool.tile([P, SQ], BF16, tag="pt", name="pt_t")
                pt_tiles.append(ptt)
                if EXP_WIDE:
                    ps = ps_sc.tile([P, SQ], F32, tag="sc", name="ps_s")
                    for sh in range(NSH):
                        nc.tensor.matmul(
                            ps[:, sh * FD:(sh + 1) * FD],
                            kt_sb[0][:, t * P:(t + 1) * P],
                            qt_sb[2 * h][:, sh * FD:(sh + 1) * FD],
                            start=True, stop=False)
                        nc.tensor.matmul(
                            ps[:, sh * FD:(sh + 1) * FD],
                            kt_sb[1][:, t * P:(t + 1) * P],
                            qt_sb[2 * h + 1][:, sh * FD:(sh + 1) * FD],
                            start=False, stop=True)
                    nc.scalar.activation(ptt, ps, AF.Exp, scale=SCALE)
                elif LDW_PAIR:
                    pss = [ps_sc.tile([P, FD], F32, tag="sc", name="ps_s")
                           for _ in range(NSH)]
                    for kj in range(2):
                        for sh in range(NSH):
                            nc.tensor.matmul(
                                pss[sh], kt_sb[kj][:, t * P:(t + 1) * P],
                                qt_sb[2 * h + kj][:, sh * FD:(sh + 1) * FD],
                                start=(kj == 0), stop=(kj == 1),
                                skip_group_check=True)
                    for sh in range(NSH):
                        nc.scalar.activation(
                            ptt[:, sh * FD:(sh + 1) * FD], pss[sh], AF.Exp, scale=SCALE)
                elif SCORES_FP8:
                    escale = SCALE / (SW * SW) if PROJ_FP8 else SCALE
                    for sh in range(NSH):
                        ps = ps_sc.tile([P, FD], F32, tag="sc", name="ps_s")
                        nc.tensor.matmul(
                            ps, kt8_sb[:, :, t * P:(t + 1) * P],
                            qt8_sb[h][:, :, sh * FD:(sh + 1) * FD],
                            start=True, stop=not QR_CHAIN,
                            perf_mode=mybir.MatmulPerfMode.DoubleRow)
                        if QR_CHAIN:
                            nc.tensor.matmul(
                                ps, kt8_sb[:, :, t * P:(t + 1) * P],
                                qtr8_sb[h][:, :, sh * FD:(sh + 1) * FD],
                                start=False, stop=True,
                                perf_mode=mybir.MatmulPerfMode.DoubleRow)
                        nc.scalar.activation(
                            ptt[:, sh * FD:(sh + 1) * FD], ps, AF.Exp, scale=escale)
                else:
                    for sh in range(NSH):
                        ps = ps_sc.tile([P, FD], F32, tag="sc", name="ps_s")
                        nc.tensor.matmul(
                            ps, kt_sb[0][:, t * P:(t + 1) * P],
                            qt_sb[2 * h][:, sh * FD:(sh + 1) * FD],
                            start=True, stop=False)
                        nc.tensor.matmul(
                            ps, kt_sb[1][:, t * P:(t + 1) * P],
                            qt_sb[2 * h + 1][:, sh * FD:(sh + 1) * FD],
                            start=False, stop=True)
                        nc.scalar.activation(
                            ptt[:, sh * FD:(sh + 1) * FD], ps, AF.Exp, scale=SCALE)
                if t == 0:
                    nc.vector.tensor_copy(lpart, ptt)
                else:
                    nc.vector.tensor_add(lpart, lpart, ptt)

            # unnormalised attention output: O~^T[d, s] += Vc[t,d]^T P^T[t,s]
            if ATTNV_INTERLEAVE:
                # All four (sh, d-half) accumulators run in one t loop so
                # each PT tile is fully consumed at iteration t.
                ov_pairs = [
                    (ps_ov.tile([P, FD], F32, tag="ov", name="ps_ov0"),
                     ps_ov.tile([P, FD], F32, tag="ov", name="ps_ov1"))
                    for _ in range(NSH)
                ]
                for t in range(NT):
                    for dh in range(2):
                        for sh in range(NSH):
                            nc.tensor.matmul(
                                ov_pairs[sh][dh], vc_sb[t][:, dh * P:(dh + 1) * P],
                                pt_tiles[t][:, sh * FD:(sh + 1) * FD],
                                start=(t == 0), stop=(t == NT - 1))
            elif LDW_PAIR:
                ov_pairs = [
                    (ps_ov.tile([P, FD], F32, tag="ov", name="ps_ov0"),
                     ps_ov.tile([P, FD], F32, tag="ov", name="ps_ov1"))
                    for _ in range(NSH)
                ]
                for t in range(NT):
                    for dh in range(2):
                        for sh in range(NSH):
                            nc.tensor.matmul(
                                ov_pairs[sh][dh], vc_sb[t][:, dh * P:(dh + 1) * P],
                                pt_tiles[t][:, sh * FD:(sh + 1) * FD],
                                start=(t == 0), stop=(t == NT - 1),
                                skip_group_check=True)
            else:
                # One (sh) pair at a time: 2 live accumulators, 4 bufs ->
                # the pool double-buffers across s-chunks and heads.
                ov_pairs = []
                for sh in range(NSH):
                    ov0 = ps_ov.tile([P, FD], F32, tag="ov", name="ps_ov0")
                    ov1 = ps_ov.tile([P, FD], F32, tag="ov", name="ps_ov1")
                    ov_pairs.append((ov0, ov1))
                    for t in range(NT):
                        nc.tensor.matmul(
                            ov0, vc_sb[t][:, 0:P],
                            pt_tiles[t][:, sh * FD:(sh + 1) * FD],
                            start=(t == 0), stop=(t == NT - 1))
                        nc.tensor.matmul(
                            ov1, vc_sb[t][:, P:DKV],
                            pt_tiles[t][:, sh * FD:(sh + 1) * FD],
                            start=(t == 0), stop=(t == NT - 1))

            # l = sum_t P^T[t, s] (partition sum via ones matmul), rl = 1/l,
            # RL = broadcast of rl over 128 partitions (rank-1 matmul).
            rlb = lpool.tile([P, SQ], F32, tag="rlb", name="rlb")
            if L_VIA_GPSIMD:
                # GPSIMD does the partition sum AND the broadcast in one op,
                # freeing the PE matmuls and the l/RL PSUM bank.
                lbc = lpool.tile([P, SQ], F32, tag="lbc", name="lbc")
                nc.gpsimd.partition_all_reduce(
                    lbc, lpart, channels=P, reduce_op=bass_isa.ReduceOp.add)
                nc.vector.reciprocal(rlb, lbc)
            elif LRL_SPLIT:
                # one-bank l/RL pipeline, processed per s-chunk
                rl_row = lpool.tile([1, SQ], F32, tag="rl_row", name="rl_row")
                lrl_pool = {"lrl": ps_lrl if LRL_POOL == "lrl" else None,
                            "ov": ps_ov, "sc": ps_sc}[LRL_POOL]
                for sh in range(NSH):
                    l_ps = lrl_pool.tile([1, FD], F32, tag="sc" if LRL_POOL != "lrl" else "lrl", name="ps_l")
                    nc.tensor.matmul(
                        l_ps, ones_col, lpart[:, sh * FD:(sh + 1) * FD],
                        start=True, stop=True)
                    if RECIP_VIA_ACT:
                        # 1/l = exp(-ln l): both funcs live in the same ACT
                        # table set as the softmax exp -> no table swaps, and
                        # ~6x faster than the DVE iterative divide.
                        lnl = lpool.tile([1, FD], F32, tag="lnl", name="lnl")
                        nc.scalar.activation(lnl, l_ps, AF.Ln)
                        nc.scalar.activation(
                            rl_row[:, sh * FD:(sh + 1) * FD], lnl, AF.Exp,
                            scale=-1.0)
                    else:
                        nc.vector.reciprocal(rl_row[:, sh * FD:(sh + 1) * FD], l_ps)
                    rl_ps = lrl_pool.tile([P, FD], F32, tag="sc" if LRL_POOL != "lrl" else "lrl", name="ps_rl")
                    nc.tensor.matmul(
                        rl_ps, ones_row, rl_row[:, sh * FD:(sh + 1) * FD],
                        start=True, stop=True)
                    nc.scalar.activation(rlb[:, sh * FD:(sh + 1) * FD], rl_ps, AF.Copy)
            else:
                l_ps = ps_lrl.tile([1, SQ], F32, tag="lrl", name="ps_l")
                for sh in range(NSH):
                    nc.tensor.matmul(
                        l_ps[:, sh * FD:(sh + 1) * FD], ones_col,
                        lpart[:, sh * FD:(sh + 1) * FD], start=True, stop=True)
                rl_row = lpool.tile([1, SQ], F32, tag="rl_row", name="rl_row")
                nc.vector.reciprocal(rl_row, l_ps)
                rl_ps = ps_lrl.tile([P, SQ], F32, tag="lrl", name="ps_rl")
                for sh in range(NSH):
                    nc.tensor.matmul(
                        rl_ps[:, sh * FD:(sh + 1) * FD], ones_row,
                        rl_row[:, sh * FD:(sh + 1) * FD], start=True, stop=True)
                nc.scalar.activation(rlb, rl_ps, AF.Copy)

            # normalise while copying PSUM -> SBUF (bf16 for the W_o matmul)
            for sh in range(NSH):
                ov0, ov1 = ov_pairs[sh]
                nc.vector.tensor_mul(
                    ot_sb[2 * h][:, sh * FD:(sh + 1) * FD], ov0,
                    rlb[:, sh * FD:(sh + 1) * FD])
                nc.vector.tensor_mul(
                    ot_sb[2 * h + 1][:, sh * FD:(sh + 1) * FD], ov1,
                    rlb[:, sh * FD:(sh + 1) * FD])

        # ---- phase W: Y = O @ W_o^T ----
        for sb in range(SQ // P):
            ysb = ypool.tile([P, DM], F32, tag="y", name="ysb")
            for ec in range(DM // FD):
                ps = ps_sc.tile([P, FD], F32, tag="sc", name="ps_y")
                for j in range(NCT):
                    nc.tensor.matmul(
                        ps, ot_sb[j][:, sb * P:(sb + 1) * P],
                        wot_sb[j][:, ec * FD:(ec + 1) * FD],
                        start=(j == 0), stop=(j == NCT - 1))
                nc.scalar.activation(ysb[:, ec * FD:(ec + 1) * FD], ps, AF.Copy)
                if EDGE_TRIM_TAIL:
                    nc.sync.dma_start(
                        out=y[sb * P:(sb + 1) * P, ec * FD:(ec + 1) * FD],
                        in_=ysb[:, ec * FD:(ec + 1) * FD])
            if not EDGE_TRIM_TAIL:
                nc.sync.dma_start(out=y[sb * P:(sb + 1) * P, :], in_=ysb)


_BUILD_CACHE = {}


def build_program(use_cc=USE_CC, reps=1):
    """Build + compile the per-core Bass program (cached per process)."""
    key = ("nc", use_cc, reps)
    if key in _BUILD_CACHE:
        return _BUILD_CACHE[key]
    nc = bacc.Bacc("TRN2", target_bir_lowering=False, debug=False,
                   num_devices=NCORES)
    extra = ()
    if PROJ_FP8:
        assert SCORES_FP8 and not use_cc
        xt = xq = wqt = wkt = wvt = None
        extra = tuple(
            nc.dram_tensor(nm, shp, F8, kind="ExternalInput").ap()
            for nm, shp in (
                ("xq8", [DM, SQ]), ("xqr", [DM, SQ]),
                ("wq8", [DM, DM]), ("wqr", [DM, DM]),
                ("wk8", [DM, DKV]), ("wkr", [DM, DKV]),
                ("wv8", [DM, DKV]), ("wvr", [DM, DKV]),
                ("xt8", [DM, S]), ("xtr", [DM, S]),
            ))
    else:
        xt = (nc.dram_tensor("xt", [DM, S], BF16, kind="ExternalInput").ap()
              if not use_cc else None)
        xq = nc.dram_tensor("xq", [DM, SQ], BF16, kind="ExternalInput").ap()
        wqt = nc.dram_tensor("wqt", [DM, DM], BF16, kind="ExternalInput").ap()
        wkt = nc.dram_tensor("wkt", [DM, DKV], BF16, kind="ExternalInput").ap()
        wvt = nc.dram_tensor("wvt", [DM, DKV], BF16, kind="ExternalInput").ap()
    wot = nc.dram_tensor("wot", [DM, DM], BF16, kind="ExternalInput").ap()
    y = nc.dram_tensor("y", [SQ, DM], F32, kind="ExternalOutput").ap()
    with tile.TileContext(nc) as tc:
        for rep in range(reps):
            _emit_full(tc, (y, xt, xq, wqt, wkt, wvt, wot) + extra,
                       use_cc, rep=rep)
    nc.compile()
    _BUILD_CACHE[key] = nc
    return nc


def _split8(a):
    """fp32 array -> (fp8(a), fp8(a - fp8(a))) as float8_e4m3."""
    npf8 = mybir.dt.np(F8)
    hi = a.astype(npf8)
    lo = (a - hi.astype(np.float32)).astype(npf8)
    return hi, lo


def make_in_maps(X, W_q, W_k, W_v, W_o, use_cc=USE_CC):
    """Host-side shard prep: transpose + cast, one input dict per core."""
    wot = np.ascontiguousarray(W_o.T).astype(NPBF16)
    if PROJ_FP8:
        wq8, wqr = _split8(np.ascontiguousarray(W_q.T) * np.float32(SW))
        wk8, wkr = _split8(np.ascontiguousarray(W_k.T) * np.float32(SW))
        wv8, wvr = _split8(np.ascontiguousarray(W_v.T) * np.float32(SW))
        x8s = [_split8(np.ascontiguousarray(X[b].T)) for b in range(B)]
        in_maps = []
        for c in range(NCORES):
            b, chunk = divmod(c, CHUNKS)
            sl = slice(chunk * SQ, (chunk + 1) * SQ)
            in_maps.append({
                "xq8": np.ascontiguousarray(x8s[b][0][:, sl]),
                "xqr": np.ascontiguousarray(x8s[b][1][:, sl]),
                "wq8": wq8, "wqr": wqr, "wk8": wk8, "wkr": wkr,
                "wv8": wv8, "wvr": wvr,
                "xt8": x8s[b][0], "xtr": x8s[b][1], "wot": wot,
            })
        return in_maps
    wqt = np.ascontiguousarray(W_q.T).astype(NPBF16)
    wkt = np.ascontiguousarray(W_k.T).astype(NPBF16)
    wvt = np.ascontiguousarray(W_v.T).astype(NPBF16)
    xts = [np.ascontiguousarray(X[b].T).astype(NPBF16) for b in range(B)]
    in_maps = []
    for c in range(NCORES):
        b, chunk = divmod(c, CHUNKS)
        xq = np.ascontiguousarray(xts[b][:, chunk * SQ:(chunk + 1) * SQ])
        m = {"xq": xq, "wqt": wqt, "wkt": wkt, "wvt": wvt, "wot": wot}
        if not use_cc:
            m["xt"] = xts[b]
        in_maps.append(m)
    return in_maps


def _run_once(X, W_q, W_k, W_v, W_o, trace, trace_cores, use_cc):
    nc = build_program(use_cc)
    in_maps = make_in_maps(X, W_q, W_k, W_v, W_o, use_cc)
    res = run_bass_kernel_spmd(
        nc, in_maps, list(range(NCORES)), trace=trace, trace_cores=trace_cores)
    Y = np.empty((B, S, DM), np.float32)
    for c in range(NCORES):
        b, chunk = divmod(c, CHUNKS)
        Y[b, chunk * SQ:(chunk + 1) * SQ, :] = res.results[c]["y"]
    return Y, res


def run(X, W_q, W_k, W_v, W_o, trace=False, trace_cores=None, use_cc=USE_CC):
    """Run the 8-core kernel in-process; returns (Y, BassKernelResults)."""
    return _run_once(X, W_q, W_k, W_v, W_o, trace, trace_cores, use_cc)


def _try_cc_subprocess(X, W_q, W_k, W_v, W_o):
    """Run the collective (use_cc) variant in an isolated child process.

    The AllGather path intermittently (~2/10 fresh sessions) kills the NRT
    worker ("notify failed ... hung up"), and that poisons the whole
    process's PJRT backend -- an in-process fallback cannot recover.  A child
    process isolates the blast radius: if it dies, this (parent) process has
    never attached to the devices and can run the collective-free variant
    cleanly.  Returns Y, or None on any child failure.
    """
    import os
    import subprocess
    import sys
    import tempfile

    d = tempfile.mkdtemp(prefix="mla_cc_")
    inp = os.path.join(d, "in.npz")
    outp = os.path.join(d, "out.npy")
    np.savez(inp, X=X, W_q=W_q, W_k=W_k, W_v=W_v, W_o=W_o)
    code = (
        "import sys, numpy as np\n"
        f"sys.path.insert(0, {os.path.dirname(os.path.abspath(__file__))!r})\n"
        "import kernel\n"
        f"z = np.load({inp!r})\n"
        "Y, _ = kernel.run(z['X'], z['W_q'], z['W_k'], z['W_v'], z['W_o'],"
        " use_cc=True)\n"
        f"np.save({outp!r}, Y)\n"
    )
    try:
        proc = subprocess.run([sys.executable, "-c", code], timeout=2400,
                              capture_output=True)
        if proc.returncode != 0 or not os.path.exists(outp):
            sys.stderr.write(
                "kernel: collective subprocess failed "
                f"(rc={proc.returncode}); falling back to no-CC\n"
                + proc.stderr.decode(errors="replace")[-2000:] + "\n")
            return None
        return np.load(outp)
    except Exception as e:
        sys.stderr.write(f"kernel: collective subprocess error {e!r}; "
                         "falling back to no-CC\n")
        return None


def kernel(X, W_q, W_k, W_v, W_o):
    X = np.asarray(X, np.float32)
    W_q = np.asarray(W_q, np.float32)
    W_k = np.asarray(W_k, np.float32)
    W_v = np.asarray(W_v, np.float32)
    W_o = np.asarray(W_o, np.float32)
    if USE_CC:
        Y = _try_cc_subprocess(X, W_q, W_k, W_v, W_o)
        if Y is not None:
            return Y
    Y, _ = run(X, W_q, W_k, W_v, W_o, use_cc=False)
    return Y

